# revision 1
# baseline (speedup 1.0000x reference)
"""DeepSAT GNN message-passing kernel for 8 Trainium2 NeuronCores.

Algorithm notes (validated numerically against the reference):
  - Every node is updated exactly once, at step l = forward_level (levels
    1..19; level-0 nodes keep h0 forever). At update time the node's own
    hidden state is still h0, so the GRU "hidden side" gates are constant
    vectors computable on the host.
  - msg_i = W @ (S_i + n0_i*h0) + deg_i*b, where S_i sums h over "active"
    in-edges (source level in [1, level_i)), n0_i counts inactive in-edges
    and deg_i all in-edges. With u = W^-1 b this folds to
    msg_i = W @ S'_i,  S'_i = S_i + n0_i*h0 + deg_i*u, so the per-gate
    input is  gi_g = (wih_g @ W) @ S'_i + bih_g  -- one fused matmul.
  - Nodes are stored level-sorted ("rank" order): per-level writes are
    contiguous, per-level ownership is an even 8-way split, and the
    AllGather of each level's new h lands in place.

Device schedule per level l (SPMD on 8 cores):
  gather h[src] for this level's "fresh" chunks (src level == l-1),
  segment-sum via one-hot matmuls into PSUM (seeded with the n0/deg
  terms), fused GRU, PE-transpose, DMA to the AllGather input, AllGather
  into the replicated h_store, then (overlapping the collective) the MLP
  head for this level plus the next level's "old" chunks (src level
  <= l-1), whose gather only reads rows below this level's slab.
"""

import os
import sys
import numpy as np

sys.path.insert(0, "/opt/trn_rl_repo")

P = 128
D = 128
NC = 8
GW = 512  # psum group width (one bank of fp32)

_COMPILED = {}


# ---------------------------------------------------------------------------
# Host-side preprocessing
# ---------------------------------------------------------------------------

def _preprocess(forward_level, edge_index, num_levels):
    fl = np.asarray(forward_level).astype(np.int64)
    ei = np.asarray(edge_index).astype(np.int64)
    src, dst = ei[0], ei[1]
    N = fl.shape[0]
    NL = num_levels

    # --- rank space: nodes sorted by level, each level padded to NC*P ---
    n_l = np.bincount(fl, minlength=NL).astype(np.int64)
    pad_l = ((n_l + NC * P - 1) // (NC * P)) * (NC * P)
    pad_l = np.maximum(pad_l, NC * P)  # at least one block per core
    L_off = np.zeros(NL + 1, np.int64)
    L_off[1:] = np.cumsum(pad_l)
    Vc = (pad_l // NC).astype(np.int64)          # per-core nodes per level
    Voff = np.zeros(NL + 1, np.int64)
    Voff[1:] = np.cumsum(Vc)                     # per-core rank-space offsets
    nblk = (Vc // P).astype(np.int64)

    order = np.argsort(fl, kind="stable")
    starts_real = np.zeros(NL + 1, np.int64)
    starts_real[1:] = np.cumsum(n_l)
    pos_within = np.arange(N, dtype=np.int64) - starts_real[fl[order]]
    rank = np.empty(N, np.int64)
    rank[order] = L_off[fl[order]] + pos_within

    node_of_rank = np.full(L_off[NL], -1, np.int64)
    node_of_rank[rank] = np.arange(N, dtype=np.int64)

    # --- per-node degree stats, indexed by rank ---
    lv_s, lv_d = fl[src], fl[dst]
    act = (lv_s >= 1) & (lv_s < lv_d)
    deg = np.bincount(dst, minlength=N).astype(np.float64)
    n0 = np.bincount(dst[~act], minlength=N).astype(np.float64)

    sumVc = int(Voff[NL])
    n0row = np.zeros((NC, sumVc), np.float32)
    degrow = np.zeros((NC, sumVc), np.float32)
    for c in range(NC):
        # global ranks owned by core c, in per-core rank-space order
        grs = []
        for l in range(NL):
            grs.append(L_off[l] + c * Vc[l] + np.arange(Vc[l]))
        gr = np.concatenate(grs)
        nd = node_of_rank[gr]
        m = nd >= 0
        n0row[c, m] = n0[nd[m]]
        degrow[c, m] = deg[nd[m]]

    # --- active edge table ---
    er = np.where(act)[0]
    e_lvl = lv_d[er]
    e_srcrank = rank[src[er]].astype(np.int64)
    e_dstrank = rank[dst[er]].astype(np.int64)
    e_local = e_dstrank - L_off[e_lvl]
    e_core = e_local // Vc[e_lvl]
    e_wl = e_local % Vc[e_lvl]
    e_blk = e_wl // P
    e_lr = (e_wl % P).astype(np.int64)
    e_fresh = lv_s[er] == (e_lvl - 1)

    # group edges per (level, phase, block, core); chunk counts are the max
    # over cores so the SPMD program is identical everywhere.
    levels = []
    col = 0
    idx_cols = [[] for _ in range(NC)]   # each entry: [128] int32 src ranks
    rank_cols = [[] for _ in range(NC)]  # each entry: [128] f32 local ranks
    for l in range(NL):
        info = {
            "old_chunks": [],    # (grp, blk_in_grp, col)
            "fresh_chunks": [],
            "old_cols": (col, 0),
            "fresh_cols": (col, 0),
        }
        if l >= 1:
            in_lvl = e_lvl == l
            for phase, sel0 in (("old", in_lvl & ~e_fresh), ("fresh", in_lvl & e_fresh)):
                phase_start = col
                chunks = []
                for blk in range(int(nblk[l])):
                    sel_b = sel0 & (e_blk == blk)
                    percore = [np.where(sel_b & (e_core == c))[0] for c in range(NC)]
                    counts = [len(x) for x in percore]
                    nch = (max(counts) + P - 1) // P
                    for ch in range(nch):
                        for c in range(NC):
                            es = percore[c][ch * P:(ch + 1) * P]
                            iv = np.zeros(P, np.int32)
                            rv = np.full(P, -1.0, np.float32)
                            iv[: len(es)] = e_srcrank[es]
                            rv[: len(es)] = e_lr[es]
                            idx_cols[c].append(iv)
                            rank_cols[c].append(rv)
                        chunks.append((blk // 4, blk % 4, col))
                        col += 1
                info[phase + "_chunks"] = chunks
                info[phase + "_cols"] = (phase_start, col - phase_start)
        levels.append(info)

    TC = max(col, 1)
    idxs = np.zeros((NC, P, TC), np.int32)
    ranks = np.full((NC, P, TC), -1.0, np.float32)
    for c in range(NC):
        if idx_cols[c]:
            idxs[c, :, :col] = np.stack(idx_cols[c], axis=1)
            ranks[c, :, :col] = np.stack(rank_cols[c], axis=1)

    # stop-flag bookkeeping: last matmul per (level, grp)
    for l in range(NL):
        info = levels[l]
        ngrp = (int(Vc[l]) + GW - 1) // GW
        last = {}
        for name in ("old_chunks", "fresh_chunks"):
            for (grp, big, c0) in info[name]:
                last[grp] = (name, c0)
        info["ngrp"] = ngrp
        info["last"] = last

    return {
        "N": N, "NL": NL, "n_l": n_l, "pad": pad_l, "L_off": L_off,
        "Vc": Vc, "Voff": Voff, "nblk": nblk, "sumVc": sumVc, "TC": TC,
        "levels": levels, "idxs": idxs, "ranks": ranks,
        "n0row": n0row, "degrow": degrow, "node_of_rank": node_of_rank,
    }


def _prep_weights(inp):
    f64 = np.float64
    W = inp["aggr_w"].astype(f64)
    b = inp["aggr_b"].astype(f64)
    h0 = (inp["emd_w"][:, 0] + inp["emd_b"]).astype(f64)
    wih = inp["gru_wih"].astype(f64)
    whh = inp["gru_whh"].astype(f64)
    bih = inp["gru_bih"].astype(f64)
    bhh = inp["gru_bhh"].astype(f64)
    u = np.linalg.solve(W, b)
    assert np.abs(W @ u - b).max() < 1e-6
    ghc = whh @ h0 + bhh
    hr_c, hz_c, hn_c = ghc[:D], ghc[D:2 * D], ghc[2 * D:]
    bih_r, bih_z, bih_n = bih[:D], bih[D:2 * D], bih[2 * D:]
    WgT = [(wih[g * D:(g + 1) * D] @ W).T for g in range(3)]

    W1 = inp["w1"].astype(f64)  # [256, 128]
    W2 = inp["w2"].astype(f64)  # [256, 256]
    w3 = inp["w3"].astype(f64)  # [1, 256]
    DM = W1.shape[0]
    assert DM == 256

    blocks = [
        WgT[0], WgT[1], WgT[2], np.diag(hn_c),
        W1[0:128, :].T, W1[128:256, :].T,
        W2[0:128, 0:128].T, W2[0:128, 128:256].T,
        W2[128:256, 0:128].T, W2[128:256, 128:256].T,
        np.eye(128), np.tile(np.arange(128, dtype=f64)[None, :], (128, 1)),
    ]
    wmat = np.concatenate(blocks, axis=1).astype(np.float32)  # [128, 12*128]

    vcols = np.stack([
        h0,                      # 0: h0 column
        bih_r + hr_c,            # 1: sigmoid bias for r
        -(bih_z + hz_c),         # 2: sigmoid bias for z' (scale = -1)
        bih_n,                   # 3: tanh bias for n
        inp["b1"].astype(f64)[0:128],    # 4
        inp["b1"].astype(f64)[128:256],  # 5
        inp["b2"].astype(f64)[0:128],    # 6
        inp["b2"].astype(f64)[128:256],  # 7
        w3[0, 0:128],            # 8
        w3[0, 128:256],          # 9
        np.full(128, inp["b3"].astype(f64)[0]),  # 10: b3 (row 0 used)
    ], axis=1).astype(np.float32)  # [128, 11]

    vrow = np.zeros((1, 256), np.float32)
    vrow[0, :128] = h0.astype(np.float32)
    vrow[0, 128:] = u.astype(np.float32)
    return wmat, vcols, vrow


# ---------------------------------------------------------------------------
# Bass program
# ---------------------------------------------------------------------------

WM = {name: i for i, name in enumerate(
    ["WgT_r", "WgT_z", "WgT_n", "diag_hn", "W1Ta", "W1Tb",
     "W2_k0m0", "W2_k1m0", "W2_k0m1", "W2_k1m1", "ident", "iota"])}
VC = {name: i for i, name in enumerate(
    ["h0", "bias_r", "nbias_z", "bias_n", "b1a", "b1b", "b2a", "b2b",
     "w3a", "w3b", "b3"])}


def _build(sched, reps=1):
    import concourse.bacc as bacc
    import concourse.tile as tile
    from concourse import bass, mybir

    f32 = mybir.dt.float32
    i32 = mybir.dt.int32
    AF = mybir.ActivationFunctionType
    OP = mybir.AluOpType
    NL = sched["NL"]
    L_off = sched["L_off"]
    Vc = sched["Vc"]
    Voff = sched["Voff"]
    nblk = sched["nblk"]
    pad = sched["pad"]
    TC = sched["TC"]
    sumVc = sched["sumVc"]
    NpadTot = int(L_off[NL])
    RG = [list(range(NC))]

    nc = bacc.Bacc("TRN2", target_bir_lowering=False, debug=False,
                   enable_asserts=False, num_devices=NC)

    wmat_d = nc.dram_tensor("wmat", [P, P * len(WM)], f32, kind="ExternalInput")
    vcols_d = nc.dram_tensor("vcols", [P, len(VC)], f32, kind="ExternalInput")
    vrow_d = nc.dram_tensor("vrow", [1, 256], f32, kind="ExternalInput")
    n0_d = nc.dram_tensor("n0row", [1, sumVc], f32, kind="ExternalInput")
    deg_d = nc.dram_tensor("degrow", [1, sumVc], f32, kind="ExternalInput")
    idx_d = nc.dram_tensor("idxs", [P, TC], i32, kind="ExternalInput")
    rnk_d = nc.dram_tensor("ranks", [P, TC], f32, kind="ExternalInput")
    pred_d = nc.dram_tensor("pred", [sumVc], f32, kind="ExternalOutput")
    h_store = nc.dram_tensor("h_store", [NpadTot, D], f32, kind="Internal",
                             addr_space="Shared")
    ag_in = [nc.dram_tensor(f"ag_in{i}", [int(Vc.max()), D], f32, kind="Internal")
             for i in range(2)]

    with tile.TileContext(nc) as tc:
        cpool = tc.alloc_tile_pool(name="const", bufs=1)
        spool = tc.alloc_tile_pool(name="sbuf", bufs=2)
        gpool = tc.alloc_tile_pool(name="gath", bufs=2)
        hpool = tc.alloc_tile_pool(name="hnew", bufs=6)
        ppool = tc.alloc_tile_pool(name="psS", bufs=3, space="PSUM")
        qpool = tc.alloc_tile_pool(name="psG", bufs=3, space="PSUM")
        tpool = tc.alloc_tile_pool(name="psT", bufs=1, space="PSUM")
        rpool = tc.alloc_tile_pool(name="psP", bufs=1, space="PSUM")

        # ---- load constants ----
        wm = cpool.tile([P, P * len(WM)], f32, tag="wm")
        nc.sync.dma_start(out=wm[:], in_=wmat_d[:])
        vc = cpool.tile([P, len(VC)], f32, tag="vc")
        nc.sync.dma_start(out=vc[:], in_=vcols_d[:])
        vr = cpool.tile([1, 256], f32, tag="vr")
        nc.sync.dma_start(out=vr[:], in_=vrow_d[:])
        idxs = cpool.tile([P, TC], i32, tag="idxs")
        nc.sync.dma_start(out=idxs[:], in_=idx_d[:])
        rnks = cpool.tile([P, TC], f32, tag="rnks")
        nc.sync.dma_start(out=rnks[:], in_=rnk_d[:])

        def wmb(name):
            return wm[:, WM[name] * P:(WM[name] + 1) * P]

        def vcc(name):
            return vc[:, VC[name]:VC[name] + 1]

        h0b = cpool.tile([P, GW], f32, tag="h0b")  # h0 broadcast along free
        nc.vector.tensor_copy(out=h0b[:], in_=vcc("h0").to_broadcast([P, GW]))

        # zero h_store row 0 (dummy gather target for padded edge lanes)
        zrow = cpool.tile([1, D], f32, tag="zrow")
        nc.vector.memset(zrow[:], 0.0)
        zrow_dma = nc.sync.dma_start(out=h_store[0:1, :], in_=zrow[:])

        # ---- per-level state ----
        S_ps = [None] * (NL)       # list of psum tiles per level (by grp)
        Hg_old = [None] * (NL + 1)
        last_ag = [None]           # most recent AllGather instruction

        def grp_widths(l):
            ws = []
            v = int(Vc[l])
            while v > 0:
                ws.append(min(GW, v))
                v -= GW
            return ws

        def emit_onehot(info, which):
            c0, k = info[which + "_cols"]
            if k == 0:
                return None
            oh = spool.tile([P, k * P], f32, tag="oh")
            CH = 4
            for s in range(0, k, CH):
                m = min(CH, k - s)
                nc.vector.tensor_tensor(
                    out=oh[:, s * P:(s + m) * P].rearrange("p (m f) -> p m f", m=m),
                    in0=rnks[:, c0 + s:c0 + s + m][:, :, None].to_broadcast([P, m, P]),
                    in1=wmb("iota")[:, None, :].to_broadcast([P, m, P]),
                    op=OP.is_equal,
                )
            return oh

        def emit_gather(info, which, bound_level):
            """Gather h rows for one phase; reads only h_store[0:L_off[bound]].

            The prefix bound keeps the read region disjoint from AllGathers
            of levels >= bound_level, so Tile lets them overlap.
            """
            c0, k = info[which + "_cols"]
            if k == 0:
                return None
            hg = gpool.tile([P, k * D], f32, tag="hg_" + which)
            # the HW indirect DMA consumes ONE index per partition (the
            # interpreter's multi-column semantics do not match silicon), so
            # issue one gather per 128-edge chunk.
            for j in range(k):
                gi = nc.gpsimd.indirect_dma_start(
                    out=hg[:, j * D:(j + 1) * D],
                    out_offset=None,
                    in_=h_store[0:int(L_off[bound_level]), :],
                    in_offset=bass.IndirectOffsetOnAxis(
                        ap=idxs[:, c0 + j:c0 + j + 1], axis=0),
                )
                # the dynamic read of h_store is not region-tracked by Tile's
                # shadow memory: pin the RAW edges by hand
                if last_ag[0] is not None:
                    tile.add_dep_helper(gi.ins, last_ag[0].ins, sync=True,
                                        reason="gather reads AllGather output")
                tile.add_dep_helper(gi.ins, zrow_dma.ins, sync=True,
                                    reason="gather may read zeroed row 0")
            return hg

        def emit_seeds(l):
            """allocate S psums for level l and seed with n0*h0 + deg*u."""
            tiles = []
            info = sched["levels"][l]
            v = int(Vc[l])
            off = int(Voff[l])
            n0r = spool.tile([1, int(Vc.max())], f32, tag="n0r")
            nc.sync.dma_start(out=n0r[0:1, :v], in_=n0_d[0:1, off:off + v])
            degr = spool.tile([1, int(Vc.max())], f32, tag="degr")
            nc.sync.dma_start(out=degr[0:1, :v], in_=deg_d[0:1, off:off + v])
            for g, w in enumerate(grp_widths(l)):
                sp = ppool.tile([P, GW], f32, tag="S", space="PSUM")
                nc.tensor.matmul(
                    out=sp[:, :w], lhsT=vr[0:1, 0:128],
                    rhs=n0r[0:1, g * GW:g * GW + w],
                    start=True, stop=False, skip_group_check=True)
                is_last = info["last"].get(g) is None
                nc.tensor.matmul(
                    out=sp[:, :w], lhsT=vr[0:1, 128:256],
                    rhs=degr[0:1, g * GW:g * GW + w],
                    start=False, stop=is_last, skip_group_check=True)
                tiles.append(sp)
            S_ps[l] = tiles

        def emit_chunks(l, which, hg, oh):
            info = sched["levels"][l]
            chunks = info[which + "_chunks"]
            if not chunks:
                return
            c0 = info[which + "_cols"][0]
            for (grp, big, col) in chunks:
                j = col - c0
                is_last = info["last"].get(grp) == (which + "_chunks", col)
                nc.tensor.matmul(
                    out=S_ps[l][grp][:, big * P:(big + 1) * P],
                    lhsT=hg[:, j * D:(j + 1) * D],
                    rhs=oh[:, j * P:(j + 1) * P],
                    start=False, stop=is_last, skip_group_check=True)

        def emit_mlp(l, g, w, rhs_sb):
            """MLP head for one 512-group; writes pred rows."""
            z1s = []
            for half in ("a", "b"):
                zp = qpool.tile([P, GW], f32, tag="G", space="PSUM")
                nc.tensor.matmul(out=zp[:, :w], lhsT=wmb("W1T" + half),
                                 rhs=rhs_sb[:, :w], start=True, stop=True)
                zs = spool.tile([P, GW], f32, tag="z1" + half)
                nc.scalar.activation(out=zs[:, :w], in_=zp[:, :w], func=AF.Relu,
                                     bias=vcc("b1" + half))
                z1s.append(zs)
            z2s = []
            for mi, mh in enumerate(("m0", "m1")):
                zp = qpool.tile([P, GW], f32, tag="G", space="PSUM")
                nc.tensor.matmul(out=zp[:, :w], lhsT=wmb("W2_k0" + mh),
                                 rhs=z1s[0][:, :w], start=True, stop=False)
                nc.tensor.matmul(out=zp[:, :w], lhsT=wmb("W2_k1" + mh),
                                 rhs=z1s[1][:, :w], start=False, stop=True)
                zs = spool.tile([P, GW], f32, tag="z2" + mh)
                nc.scalar.activation(out=zs[:, :w], in_=zp[:, :w], func=AF.Relu,
                                     bias=vcc("b2" + ("a" if mi == 0 else "b")))
                z2s.append(zs)
            pp = rpool.tile([1, GW], f32, tag="pred", space="PSUM")
            nc.tensor.matmul(out=pp[:, :w], lhsT=vcc("w3a"), rhs=z2s[0][:, :w],
                             start=True, stop=False)
            nc.tensor.matmul(out=pp[:, :w], lhsT=vcc("w3b"), rhs=z2s[1][:, :w],
                             start=False, stop=True)
            ps = spool.tile([1, GW], f32, tag="psb")
            nc.scalar.activation(out=ps[:, :w], in_=pp[:, :w], func=AF.Identity,
                                 bias=vc[0:1, VC["b3"]:VC["b3"] + 1])
            off = int(Voff[l]) + g * GW
            nc.sync.dma_start(out=pred_d[off:off + w], in_=ps[0:1, :w])

        # reps>1 repeats the whole computation for wall-clock timing: the
        # computation is idempotent (h_store/pred rewritten with same values)
        for _rep in range(reps):
          # ================= level 0: MLP on h0 only =================
          for g, w in enumerate(grp_widths(0)):
            emit_mlp(0, g, w, h0b)

          # seeds + (no old chunks possible) for level 1
          emit_seeds(1)

          # ================= levels 1..NL-1 =================
          for l in range(1, NL):
            info = sched["levels"][l]
            widths = grp_widths(l)

            # fresh gather + chunks for this level
            hg_f = emit_gather(info, "fresh", l)
            oh_f = emit_onehot(info, "fresh")
            emit_chunks(l, "fresh", hg_f, oh_f)

            # old gather for next level: its sources are at levels <= l-1,
            # i.e. rows below L_off[l], so it overlaps this level's AllGather
            if l + 1 < NL:
                ninfo = sched["levels"][l + 1]
                Hg_old[l + 1] = (emit_gather(ninfo, "old", l),
                                 emit_onehot(ninfo, "old"))

            # GRU per group
            hnew = []
            for g, w in enumerate(widths):
                ssb = spool.tile([P, GW], f32, tag="Ssb")
                nc.vector.tensor_copy(out=ssb[:, :w], in_=S_ps[l][g][:, :w])

                gr = qpool.tile([P, GW], f32, tag="G", space="PSUM")
                nc.tensor.matmul(out=gr[:, :w], lhsT=wmb("WgT_r"),
                                 rhs=ssb[:, :w], start=True, stop=True)
                gz = qpool.tile([P, GW], f32, tag="G", space="PSUM")
                nc.tensor.matmul(out=gz[:, :w], lhsT=wmb("WgT_z"),
                                 rhs=ssb[:, :w], start=True, stop=True)
                gn = qpool.tile([P, GW], f32, tag="G", space="PSUM")
                nc.tensor.matmul(out=gn[:, :w], lhsT=wmb("WgT_n"),
                                 rhs=ssb[:, :w], start=True, stop=False)

                rsb = spool.tile([P, GW], f32, tag="rsb")
                nc.scalar.activation(out=rsb[:, :w], in_=gr[:, :w],
                                     func=AF.Sigmoid, bias=vcc("bias_r"))
                zsb = spool.tile([P, GW], f32, tag="zsb")
                nc.scalar.activation(out=zsb[:, :w], in_=gz[:, :w],
                                     func=AF.Sigmoid, bias=vcc("nbias_z"),
                                     scale=-1.0)
                nc.tensor.matmul(out=gn[:, :w], lhsT=wmb("diag_hn"),
                                 rhs=rsb[:, :w], start=False, stop=True)
                nsb = spool.tile([P, GW], f32, tag="nsb")
                nc.scalar.activation(out=nsb[:, :w], in_=gn[:, :w],
                                     func=AF.Tanh, bias=vcc("bias_n"))

                t3 = spool.tile([P, GW], f32, tag="t3")
                nc.vector.tensor_scalar(out=t3[:, :w], in0=nsb[:, :w],
                                        scalar1=vcc("h0"), scalar2=None,
                                        op0=OP.subtract)
                t4 = spool.tile([P, GW], f32, tag="t4")
                nc.vector.tensor_tensor(out=t4[:, :w], in0=t3[:, :w],
                                        in1=zsb[:, :w], op=OP.mult)
                hn = hpool.tile([P, GW], f32, tag="hnew")
                nc.vector.tensor_scalar(out=hn[:, :w], in0=t4[:, :w],
                                        scalar1=vcc("h0"), scalar2=None,
                                        op0=OP.add)
                hnew.append(hn)

            # transpose h_new to node-major, stage, and AllGather into every
            # core's h_store (skipped for the last level: nothing reads it)
            if l < NL - 1:
                agt = ag_in[l % 2]
                for g, w in enumerate(widths):
                    tp = tpool.tile([P, GW], f32, tag="tp", space="PSUM")
                    nb = w // P
                    for b in range(nb):
                        nc.tensor.transpose(
                            out=tp[:, b * P:(b + 1) * P],
                            in_=hnew[g][:, b * P:(b + 1) * P],
                            identity=wmb("ident"))
                    tps = spool.tile([P, GW], f32, tag="tps")
                    nc.vector.tensor_copy(out=tps[:, :w], in_=tp[:, :w])
                    for b in range(nb):
                        row = g * GW + b * P
                        nc.sync.dma_start(out=agt[row:row + P, :],
                                          in_=tps[:, b * P:(b + 1) * P])
                cc = nc.gpsimd.collective_compute(
                    "AllGather", bass.mybir.AluOpType.bypass,
                    replica_groups=RG,
                    ins=[agt[0:int(Vc[l]), :].opt()],
                    outs=[h_store[int(L_off[l]):int(L_off[l]) + int(pad[l]), :].opt()],
                )
                last_ag[0] = cc

            # MLP head for this level (fills the AllGather latency)
            for g, w in enumerate(widths):
                emit_mlp(l, g, w, hnew[g])

            # seeds + old chunks for the next level (also fill the AllGather)
            if l + 1 < NL:
                emit_seeds(l + 1)
                hg_o, oh_o = Hg_old[l + 1]
                emit_chunks(l + 1, "old", hg_o, oh_o)

        for pl in (rpool, tpool, qpool, ppool, hpool, gpool, spool, cpool):
            pl.release()

    nc.compile()
    return nc


# ---------------------------------------------------------------------------
# Entry point
# ---------------------------------------------------------------------------

def _run(inputs, trace=False, reps=1):
    from concourse.bass_utils import run_bass_kernel_spmd

    fl = np.asarray(inputs["forward_level"])
    num_levels = int(fl.max()) + 1
    sched = _preprocess(fl, inputs["edge_index"], num_levels)
    wmat, vcols, vrow = _prep_weights(inputs)

    key = (sched["N"], sched["TC"], sched["sumVc"], reps,
           tuple(int(x) for x in sched["Vc"]),
           tuple((len(i["old_chunks"]), len(i["fresh_chunks"]))
                 for i in sched["levels"]))
    if key not in _COMPILED:
        _COMPILED[key] = _build(sched, reps=reps)
    nc = _COMPILED[key]

    in_maps = []
    for c in range(NC):
        in_maps.append({
            "wmat": wmat, "vcols": vcols, "vrow": vrow,
            "n0row": sched["n0row"][c][None, :],
            "degrow": sched["degrow"][c][None, :],
            "idxs": sched["idxs"][c],
            "ranks": sched["ranks"][c],
        })

    res = run_bass_kernel_spmd(nc, in_maps, core_ids=list(range(NC)),
                               trace=trace)

    NL = sched["NL"]
    L_off, Vc, Voff = sched["L_off"], sched["Vc"], sched["Voff"]
    node_of_rank = sched["node_of_rank"]
    out = np.zeros(sched["N"], np.float32)
    for c in range(NC):
        oc = res.results[c]["pred"]
        for l in range(NL):
            gr = int(L_off[l]) + c * int(Vc[l]) + np.arange(int(Vc[l]))
            nd = node_of_rank[gr]
            m = nd >= 0
            out[nd[m]] = oc[int(Voff[l]):int(Voff[l]) + int(Vc[l])][m]
    return out[:, None], res


def kernel(**inputs):
    out, _ = _run(inputs, trace=False)
    return out



# revision 3
# speedup vs baseline: 1.9062x; 1.9062x over previous
"""DeepSAT GNN message-passing kernel for 8 Trainium2 NeuronCores.

Algorithm notes (validated numerically against the reference):
  - Every node is updated exactly once, at step l = forward_level (levels
    1..19; level-0 nodes keep h0 forever). At update time the node's own
    hidden state is still h0, so the GRU "hidden side" gates are constant
    vectors computable on the host.
  - msg_i = W @ (S_i + n0_i*h0) + deg_i*b, where S_i sums h over "active"
    in-edges (source level in [1, level_i)), n0_i counts inactive in-edges
    and deg_i all in-edges. With u = W^-1 b this folds to
    msg_i = W @ S'_i,  S'_i = S_i + n0_i*h0 + deg_i*u, so the per-gate
    input is  gi_g = (wih_g @ W) @ S'_i + bih_g  -- one fused matmul.
  - Nodes are stored level-sorted ("rank" order): per-level writes are
    contiguous, per-level ownership is an even 8-way split, and the
    AllGather of each level's new h lands in place.
  - The whole h pipeline runs in bf16 (h_store, gathers, one-hot segment
    sums, GRU/MLP weights); PSUM accumulation stays fp32. Level-0 output
    (MLP of the constant h0) is computed on the host.

Device schedule per level l (SPMD on 8 cores):
  gather h[src] for this level's "fresh" chunks (src level == l-1),
  segment-sum via one-hot matmuls into PSUM (seeded with the n0/deg
  terms), fused GRU, PE-transpose, one staged DMA, AllGather into the
  replicated h_store; then (overlapping the collective) the next level's
  seeds + "old" chunks (src level <= l-1, reading rows below this
  level's slab) and this level's MLP head.
"""

import os
import sys
import numpy as np

sys.path.insert(0, "/opt/trn_rl_repo")

P = 128
D = 128
NC = 8
GW = 512  # psum group width (one bank of fp32)

_COMPILED = {}


# ---------------------------------------------------------------------------
# Host-side preprocessing
# ---------------------------------------------------------------------------

def _preprocess(forward_level, edge_index, num_levels):
    fl = np.asarray(forward_level).astype(np.int64)
    ei = np.asarray(edge_index).astype(np.int64)
    src, dst = ei[0], ei[1]
    N = fl.shape[0]
    NL = num_levels

    # --- rank space: nodes sorted by level, each level padded to NC*P ---
    n_l = np.bincount(fl, minlength=NL).astype(np.int64)
    pad_l = ((n_l + NC * P - 1) // (NC * P)) * (NC * P)
    pad_l = np.maximum(pad_l, NC * P)  # at least one block per core
    L_off = np.zeros(NL + 1, np.int64)
    L_off[1:] = np.cumsum(pad_l)
    Vc = (pad_l // NC).astype(np.int64)          # per-core nodes per level
    Voff = np.zeros(NL + 1, np.int64)
    Voff[1:] = np.cumsum(Vc)                     # per-core rank-space offsets
    nblk = (Vc // P).astype(np.int64)

    order = np.argsort(fl, kind="stable")
    starts_real = np.zeros(NL + 1, np.int64)
    starts_real[1:] = np.cumsum(n_l)
    pos_within = np.arange(N, dtype=np.int64) - starts_real[fl[order]]
    rank = np.empty(N, np.int64)
    rank[order] = L_off[fl[order]] + pos_within

    node_of_rank = np.full(L_off[NL], -1, np.int64)
    node_of_rank[rank] = np.arange(N, dtype=np.int64)

    # --- per-node degree stats, indexed by rank ---
    lv_s, lv_d = fl[src], fl[dst]
    act = (lv_s >= 1) & (lv_s < lv_d)
    deg = np.bincount(dst, minlength=N).astype(np.float64)
    n0 = np.bincount(dst[~act], minlength=N).astype(np.float64)

    sumVc = int(Voff[NL])
    n0row = np.zeros((NC, sumVc), np.float32)
    degrow = np.zeros((NC, sumVc), np.float32)
    for c in range(NC):
        # global ranks owned by core c, in per-core rank-space order
        grs = []
        for l in range(NL):
            grs.append(L_off[l] + c * Vc[l] + np.arange(Vc[l]))
        gr = np.concatenate(grs)
        nd = node_of_rank[gr]
        m = nd >= 0
        n0row[c, m] = n0[nd[m]]
        degrow[c, m] = deg[nd[m]]

    # --- active edge table ---
    er = np.where(act)[0]
    e_lvl = lv_d[er]
    e_srcrank = rank[src[er]].astype(np.int64)
    e_dstrank = rank[dst[er]].astype(np.int64)
    e_local = e_dstrank - L_off[e_lvl]
    e_core = e_local // Vc[e_lvl]
    e_wl = e_local % Vc[e_lvl]          # slot within the core's level range
    e_grp = e_wl // GW                  # psum group (512-wide)
    e_fresh = lv_s[er] == (e_lvl - 1)

    # group edges per (level, phase, psum-group); edges sorted by dst slot,
    # chunk counts are the max over cores so the SPMD program is identical
    # everywhere. Each 128-edge chunk targets ONE psum group; its one-hot
    # spans the full group width.
    levels = []
    col = 0
    idx_cols = [[] for _ in range(NC)]   # each entry: [128] int32 src ranks
    rank_cols = [[] for _ in range(NC)]  # each entry: [128] f32 local slot
    for l in range(NL):
        ngrp = (int(Vc[l]) + GW - 1) // GW
        info = {
            "old_chunks": [],    # (grp, col)
            "fresh_chunks": [],
            "old_cols": (col, 0),
            "fresh_cols": (col, 0),
            "ngrp": ngrp,
        }
        if l >= 1:
            in_lvl = e_lvl == l
            for phase, sel0 in (("old", in_lvl & ~e_fresh),
                                ("fresh", in_lvl & e_fresh)):
                phase_start = col
                chunks = []
                for g in range(ngrp):
                    sel_g = sel0 & (e_grp == g)
                    percore = [np.where(sel_g & (e_core == c))[0]
                               for c in range(NC)]
                    counts = [len(x) for x in percore]
                    nch = (max(counts) + P - 1) // P
                    for ch in range(nch):
                        for c in range(NC):
                            es = percore[c][ch * P:(ch + 1) * P]
                            iv = np.zeros(P, np.int32)
                            rv = np.full(P, -1.0, np.float32)
                            iv[: len(es)] = e_srcrank[es]
                            rv[: len(es)] = e_wl[es] - g * GW
                            idx_cols[c].append(iv)
                            rank_cols[c].append(rv)
                        chunks.append((g, col))
                        col += 1
                info[phase + "_chunks"] = chunks
                info[phase + "_cols"] = (phase_start, col - phase_start)
        levels.append(info)

    TC = max(col, 1)
    idxs = np.zeros((NC, P, TC), np.int32)
    ranks = np.full((NC, P, TC), -1.0, np.float32)
    for c in range(NC):
        if idx_cols[c]:
            idxs[c, :, :col] = np.stack(idx_cols[c], axis=1)
            ranks[c, :, :col] = np.stack(rank_cols[c], axis=1)

    # stop-flag bookkeeping: last matmul per (level, grp)
    for l in range(NL):
        info = levels[l]
        last = {}
        for name in ("old_chunks", "fresh_chunks"):
            for (grp, c0) in info[name]:
                last[grp] = (name, c0)
        info["last"] = last

    return {
        "N": N, "NL": NL, "n_l": n_l, "pad": pad_l, "L_off": L_off,
        "Vc": Vc, "Voff": Voff, "nblk": nblk, "sumVc": sumVc, "TC": TC,
        "levels": levels, "idxs": idxs, "ranks": ranks,
        "n0row": n0row, "degrow": degrow, "node_of_rank": node_of_rank,
    }


def _prep_weights(inp):
    import ml_dtypes
    bf16 = ml_dtypes.bfloat16
    f64 = np.float64
    W = inp["aggr_w"].astype(f64)
    b = inp["aggr_b"].astype(f64)
    h0 = (inp["emd_w"][:, 0] + inp["emd_b"]).astype(f64)
    wih = inp["gru_wih"].astype(f64)
    whh = inp["gru_whh"].astype(f64)
    bih = inp["gru_bih"].astype(f64)
    bhh = inp["gru_bhh"].astype(f64)
    u = np.linalg.solve(W, b)
    assert np.abs(W @ u - b).max() < 1e-6
    ghc = whh @ h0 + bhh
    hr_c, hz_c, hn_c = ghc[:D], ghc[D:2 * D], ghc[2 * D:]
    bih_r, bih_z, bih_n = bih[:D], bih[D:2 * D], bih[2 * D:]
    WgT = [(wih[g * D:(g + 1) * D] @ W).T for g in range(3)]

    W1 = inp["w1"].astype(f64)  # [256, 128]
    W2 = inp["w2"].astype(f64)  # [256, 256]
    w3 = inp["w3"].astype(f64)  # [1, 256]
    DM = W1.shape[0]
    assert DM == 256

    blocks = [
        WgT[0], WgT[1], WgT[2], np.diag(hn_c),
        W1[0:128, :].T, W1[128:256, :].T,
        W2[0:128, 0:128].T, W2[0:128, 128:256].T,
        W2[128:256, 0:128].T, W2[128:256, 128:256].T,
        np.eye(128),
    ]
    wmat = np.concatenate(blocks, axis=1).astype(bf16)  # [128, 11*128] bf16

    iota512 = np.tile(np.arange(GW, dtype=np.float32)[None, :], (P, 1))

    vcols = np.stack([
        h0,                      # 0: h0 column
        bih_r + hr_c,            # 1: sigmoid bias for r
        -(bih_z + hz_c),         # 2: sigmoid bias for z' (scale = -1)
        bih_n,                   # 3: tanh bias for n
        inp["b1"].astype(f64)[0:128],    # 4
        inp["b1"].astype(f64)[128:256],  # 5
        inp["b2"].astype(f64)[0:128],    # 6
        inp["b2"].astype(f64)[128:256],  # 7
        w3[0, 0:128],            # 8
        w3[0, 128:256],          # 9
        np.full(128, inp["b3"].astype(f64)[0]),  # 10: b3 (row 0 used)
    ], axis=1)
    vcols_f = vcols.astype(np.float32)   # [128, 11]
    vcols_b = vcols.astype(bf16)         # [128, 11]

    vrow = np.zeros((1, 256), np.float32)
    vrow[0, :128] = h0.astype(np.float32)
    vrow[0, 128:] = u.astype(np.float32)
    vrow_b = vrow.astype(bf16)

    # level-0 output: MLP(h0), identical for every level-0 node
    z1 = np.maximum(W1 @ h0 + inp["b1"].astype(f64), 0.0)
    z2 = np.maximum(W2 @ z1 + inp["b2"].astype(f64), 0.0)
    pred0 = float(w3[0] @ z2 + inp["b3"].astype(f64)[0])

    return wmat, iota512, vcols_f, vcols_b, vrow_b, pred0


# ---------------------------------------------------------------------------
# Bass program
# ---------------------------------------------------------------------------

WM = {name: i for i, name in enumerate(
    ["WgT_r", "WgT_z", "WgT_n", "diag_hn", "W1Ta", "W1Tb",
     "W2_k0m0", "W2_k1m0", "W2_k0m1", "W2_k1m1", "ident"])}
VC = {name: i for i, name in enumerate(
    ["h0", "bias_r", "nbias_z", "bias_n", "b1a", "b1b", "b2a", "b2b",
     "w3a", "w3b", "b3"])}


def _build(sched, reps=1):
    import concourse.bacc as bacc
    import concourse.tile as tile
    from concourse import bass, mybir

    f32 = mybir.dt.float32
    bf16 = mybir.dt.bfloat16
    i32 = mybir.dt.int32
    AF = mybir.ActivationFunctionType
    OP = mybir.AluOpType
    NL = sched["NL"]
    L_off = sched["L_off"]
    Vc = sched["Vc"]
    Voff = sched["Voff"]
    pad = sched["pad"]
    TC = sched["TC"]
    sumVc = sched["sumVc"]
    Vmax = int(Vc.max())
    NpadTot = int(L_off[NL])
    RG = [list(range(NC))]

    nc = bacc.Bacc("TRN2", target_bir_lowering=False, debug=False,
                   enable_asserts=False, num_devices=NC)

    wmat_d = nc.dram_tensor("wmat", [P, P * len(WM)], bf16, kind="ExternalInput")
    iota_d = nc.dram_tensor("iota512", [P, GW], f32, kind="ExternalInput")
    vcf_d = nc.dram_tensor("vcolsf", [P, len(VC)], f32, kind="ExternalInput")
    vcb_d = nc.dram_tensor("vcolsb", [P, len(VC)], bf16, kind="ExternalInput")
    vrow_d = nc.dram_tensor("vrowb", [1, 256], bf16, kind="ExternalInput")
    n0_d = nc.dram_tensor("n0row", [1, sumVc], bf16, kind="ExternalInput")
    deg_d = nc.dram_tensor("degrow", [1, sumVc], bf16, kind="ExternalInput")
    idx_d = nc.dram_tensor("idxs", [P, TC], i32, kind="ExternalInput")
    rnk_d = nc.dram_tensor("ranks", [P, TC], f32, kind="ExternalInput")
    pred_d = nc.dram_tensor("pred", [sumVc], f32, kind="ExternalOutput")
    h_store = nc.dram_tensor("h_store", [NpadTot, D], bf16, kind="Internal",
                             addr_space="Shared")
    ag_in = [nc.dram_tensor(f"ag_in{i}", [Vmax, D], bf16, kind="Internal")
             for i in range(2)]

    with tile.TileContext(nc) as tc:
        cpool = tc.alloc_tile_pool(name="const", bufs=1)
        spool = tc.alloc_tile_pool(name="sbuf", bufs=2)
        gpool = tc.alloc_tile_pool(name="gath", bufs=2)
        opool = tc.alloc_tile_pool(name="oneh", bufs=4)
        hpool = tc.alloc_tile_pool(name="hnew", bufs=6)
        ppool = tc.alloc_tile_pool(name="psS", bufs=3, space="PSUM")
        qpool = tc.alloc_tile_pool(name="psG", bufs=3, space="PSUM")
        tpool = tc.alloc_tile_pool(name="psT", bufs=1, space="PSUM")
        rpool = tc.alloc_tile_pool(name="psP", bufs=1, space="PSUM")

        # ---- load constants ----
        wm = cpool.tile([P, P * len(WM)], bf16, tag="wm")
        nc.sync.dma_start(out=wm[:], in_=wmat_d[:])
        iota = cpool.tile([P, GW], f32, tag="iota")
        nc.sync.dma_start(out=iota[:], in_=iota_d[:])
        vcf = cpool.tile([P, len(VC)], f32, tag="vcf")
        nc.sync.dma_start(out=vcf[:], in_=vcf_d[:])
        vcb = cpool.tile([P, len(VC)], bf16, tag="vcb")
        nc.sync.dma_start(out=vcb[:], in_=vcb_d[:])
        vr = cpool.tile([1, 256], bf16, tag="vr")
        nc.sync.dma_start(out=vr[:], in_=vrow_d[:])
        idxs = cpool.tile([P, TC], i32, tag="idxs")
        nc.sync.dma_start(out=idxs[:], in_=idx_d[:])
        rnks = cpool.tile([P, TC], f32, tag="rnks")
        nc.sync.dma_start(out=rnks[:], in_=rnk_d[:])

        def wmb(name):
            return wm[:, WM[name] * P:(WM[name] + 1) * P]

        def vf(name):
            return vcf[:, VC[name]:VC[name] + 1]

        def vb(name):
            return vcb[:, VC[name]:VC[name] + 1]

        # zero h_store row 0 (dummy gather target for padded edge lanes)
        zrow = cpool.tile([1, D], bf16, tag="zrow")
        nc.vector.memset(zrow[:], 0.0)
        zrow_dma = nc.sync.dma_start(out=h_store[0:1, :], in_=zrow[:])

        # ---- per-level state ----
        S_ps = [None] * NL         # list of psum tiles per level (by grp)
        Hg_old = [None] * (NL + 1)
        ccs = {}                   # level -> AllGather instruction

        def grp_widths(l):
            ws = []
            v = int(Vc[l])
            while v > 0:
                ws.append(min(GW, v))
                v -= GW
            return ws

        def emit_onehot(info, which):
            """One [128, GW] bf16 one-hot per chunk (fp32 compare)."""
            chunks = info[which + "_chunks"]
            if not chunks:
                return None
            c0 = info[which + "_cols"][0]
            ohs = []
            for (grp, col) in chunks:
                j = col - c0
                oh = opool.tile([P, GW], bf16, tag="oh" + which)
                nc.vector.tensor_tensor(
                    out=oh[:],
                    in0=rnks[:, col:col + 1].to_broadcast([P, GW]),
                    in1=iota[:],
                    op=OP.is_equal,
                )
                ohs.append(oh)
            return ohs

        def emit_gather(info, which, bound_level):
            """Gather h rows for one phase; reads only h_store[0:L_off[bound]].

            The prefix bound keeps the read region disjoint from AllGathers
            of levels >= bound_level, so Tile lets them overlap.
            """
            c0, k = info[which + "_cols"]
            if k == 0:
                return None
            dep_cc = ccs.get(bound_level - 1)
            hg = gpool.tile([P, k * D], bf16, tag="hg_" + which)
            # the HW indirect DMA consumes ONE index per partition (the
            # interpreter's multi-column semantics do not match silicon), so
            # issue one gather per 128-edge chunk.
            for j in range(k):
                gi = nc.gpsimd.indirect_dma_start(
                    out=hg[:, j * D:(j + 1) * D],
                    out_offset=None,
                    in_=h_store[0:int(L_off[bound_level]), :],
                    in_offset=bass.IndirectOffsetOnAxis(
                        ap=idxs[:, c0 + j:c0 + j + 1], axis=0),
                )
                # the dynamic read of h_store is not region-tracked by Tile's
                # shadow memory: pin the RAW edges by hand
                if dep_cc is not None:
                    tile.add_dep_helper(gi.ins, dep_cc.ins, sync=True,
                                        reason="gather reads AllGather output")
                tile.add_dep_helper(gi.ins, zrow_dma.ins, sync=True,
                                    reason="gather may read zeroed row 0")
            return hg

        def emit_seeds(l):
            """allocate S psums for level l and seed with n0*h0 + deg*u."""
            tiles = []
            info = sched["levels"][l]
            v = int(Vc[l])
            off = int(Voff[l])
            n0r = spool.tile([1, Vmax], bf16, tag="n0r")
            nc.sync.dma_start(out=n0r[0:1, :v], in_=n0_d[0:1, off:off + v])
            degr = spool.tile([1, Vmax], bf16, tag="degr")
            nc.sync.dma_start(out=degr[0:1, :v], in_=deg_d[0:1, off:off + v])
            for g, w in enumerate(grp_widths(l)):
                sp = ppool.tile([P, GW], f32, tag="S", space="PSUM")
                nc.tensor.matmul(
                    out=sp[:, :w], lhsT=vr[0:1, 0:128],
                    rhs=n0r[0:1, g * GW:g * GW + w],
                    start=True, stop=False, skip_group_check=True)
                is_last = info["last"].get(g) is None
                nc.tensor.matmul(
                    out=sp[:, :w], lhsT=vr[0:1, 128:256],
                    rhs=degr[0:1, g * GW:g * GW + w],
                    start=False, stop=is_last, skip_group_check=True)
                tiles.append(sp)
            S_ps[l] = tiles

        def emit_chunks(l, which, hg, ohs):
            info = sched["levels"][l]
            chunks = info[which + "_chunks"]
            if not chunks:
                return
            c0 = info[which + "_cols"][0]
            widths = grp_widths(l)
            for i, (grp, col) in enumerate(chunks):
                j = col - c0
                w = widths[grp]
                is_last = info["last"].get(grp) == (which + "_chunks", col)
                nc.tensor.matmul(
                    out=S_ps[l][grp][:, :w],
                    lhsT=hg[:, j * D:(j + 1) * D],
                    rhs=ohs[i][:, :w],
                    start=False, stop=is_last, skip_group_check=True)

        def emit_mlp(l, g, w, rhs_sb):
            """MLP head for one 512-group; writes pred rows."""
            z1s = []
            for half in ("a", "b"):
                zp = qpool.tile([P, GW], f32, tag="G", space="PSUM")
                nc.tensor.matmul(out=zp[:, :w], lhsT=wmb("W1T" + half),
                                 rhs=rhs_sb[:, :w], start=True, stop=True)
                zs = spool.tile([P, GW], bf16, tag="z1" + half)
                nc.vector.tensor_scalar(out=zs[:, :w], in0=zp[:, :w],
                                        scalar1=vf("b1" + half), scalar2=0.0,
                                        op0=OP.add, op1=OP.max)
                z1s.append(zs)
            z2s = []
            for mi, mh in enumerate(("m0", "m1")):
                zp = qpool.tile([P, GW], f32, tag="G", space="PSUM")
                nc.tensor.matmul(out=zp[:, :w], lhsT=wmb("W2_k0" + mh),
                                 rhs=z1s[0][:, :w], start=True, stop=False)
                nc.tensor.matmul(out=zp[:, :w], lhsT=wmb("W2_k1" + mh),
                                 rhs=z1s[1][:, :w], start=False, stop=True)
                zs = spool.tile([P, GW], bf16, tag="z2" + mh)
                nc.vector.tensor_scalar(out=zs[:, :w], in0=zp[:, :w],
                                        scalar1=vf("b2" + ("a" if mi == 0 else "b")),
                                        scalar2=0.0, op0=OP.add, op1=OP.max)
                z2s.append(zs)
            pp = rpool.tile([1, GW], f32, tag="pred", space="PSUM")
            nc.tensor.matmul(out=pp[:, :w], lhsT=vb("w3a"), rhs=z2s[0][:, :w],
                             start=True, stop=False)
            nc.tensor.matmul(out=pp[:, :w], lhsT=vb("w3b"), rhs=z2s[1][:, :w],
                             start=False, stop=True)
            ps = spool.tile([1, GW], f32, tag="psb")
            nc.scalar.activation(out=ps[:, :w], in_=pp[:, :w], func=AF.Identity,
                                 bias=vcf[0:1, VC["b3"]:VC["b3"] + 1])
            off = int(Voff[l]) + g * GW
            nc.sync.dma_start(out=pred_d[off:off + w], in_=ps[0:1, :w])

        # reps>1 repeats the whole computation for wall-clock timing: the
        # computation is idempotent (h_store/pred rewritten with same values)
        for _rep in range(reps):
          # seeds + (no old chunks possible) for level 1
          emit_seeds(1)

          # ================= levels 1..NL-1 =================
          for l in range(1, NL):
            info = sched["levels"][l]
            widths = grp_widths(l)
            v = int(Vc[l])
            nb = v // P

            # fresh gather + chunks for this level (critical path)
            hg_f = emit_gather(info, "fresh", l)
            oh_f = emit_onehot(info, "fresh")
            emit_chunks(l, "fresh", hg_f, oh_f)

            # GRU per group
            hnew = []
            for g, w in enumerate(widths):
                ssb = spool.tile([P, GW], bf16, tag="Ssb")
                nc.vector.tensor_copy(out=ssb[:, :w], in_=S_ps[l][g][:, :w])

                gr = qpool.tile([P, GW], f32, tag="G", space="PSUM")
                nc.tensor.matmul(out=gr[:, :w], lhsT=wmb("WgT_r"),
                                 rhs=ssb[:, :w], start=True, stop=True)
                gz = qpool.tile([P, GW], f32, tag="G", space="PSUM")
                nc.tensor.matmul(out=gz[:, :w], lhsT=wmb("WgT_z"),
                                 rhs=ssb[:, :w], start=True, stop=True)
                gn = qpool.tile([P, GW], f32, tag="G", space="PSUM")
                nc.tensor.matmul(out=gn[:, :w], lhsT=wmb("WgT_n"),
                                 rhs=ssb[:, :w], start=True, stop=False)

                rsb = spool.tile([P, GW], bf16, tag="rsb")
                nc.scalar.activation(out=rsb[:, :w], in_=gr[:, :w],
                                     func=AF.Sigmoid, bias=vf("bias_r"))
                zsb = spool.tile([P, GW], bf16, tag="zsb")
                nc.scalar.activation(out=zsb[:, :w], in_=gz[:, :w],
                                     func=AF.Sigmoid, bias=vf("nbias_z"),
                                     scale=-1.0)
                nc.tensor.matmul(out=gn[:, :w], lhsT=wmb("diag_hn"),
                                 rhs=rsb[:, :w], start=False, stop=True)
                nsb = spool.tile([P, GW], bf16, tag="nsb")
                nc.scalar.activation(out=nsb[:, :w], in_=gn[:, :w],
                                     func=AF.Tanh, bias=vf("bias_n"))

                t3 = spool.tile([P, GW], bf16, tag="t3")
                nc.vector.tensor_scalar(out=t3[:, :w], in0=nsb[:, :w],
                                        scalar1=vf("h0"), scalar2=None,
                                        op0=OP.subtract)
                t4 = spool.tile([P, GW], bf16, tag="t4")
                nc.vector.tensor_tensor(out=t4[:, :w], in0=t3[:, :w],
                                        in1=zsb[:, :w], op=OP.mult)
                hn = hpool.tile([P, GW], bf16, tag="hnew")
                nc.vector.tensor_scalar(out=hn[:, :w], in0=t4[:, :w],
                                        scalar1=vf("h0"), scalar2=None,
                                        op0=OP.add)
                hnew.append(hn)

            # transpose h_new to node-major, stage with ONE dma, AllGather
            # into every core's h_store (skipped for the last level)
            if l < NL - 1:
                agt = ag_in[l % 2]
                hT = spool.tile([P, Vmax], bf16, tag="hT")
                for g, w in enumerate(widths):
                    tp = tpool.tile([P, GW], bf16, tag="tp", space="PSUM")
                    for b in range(w // P):
                        nc.tensor.transpose(
                            out=tp[:, b * P:(b + 1) * P],
                            in_=hnew[g][:, b * P:(b + 1) * P],
                            identity=wmb("ident"))
                    nc.vector.tensor_copy(out=hT[:, g * GW:g * GW + w],
                                          in_=tp[:, :w])
                nc.sync.dma_start(
                    out=agt[0:v, :].rearrange("(blk p) f -> p blk f", p=P),
                    in_=hT[:, 0:v].rearrange("p (blk f) -> p blk f", f=P))
                cc = nc.gpsimd.collective_compute(
                    "AllGather", bass.mybir.AluOpType.bypass,
                    replica_groups=RG,
                    ins=[agt[0:v, :].opt()],
                    outs=[h_store[int(L_off[l]):int(L_off[l]) + int(pad[l]), :].opt()],
                )
                ccs[l] = cc

            # seeds for the next level (cheap, keeps PSUM group order), then
            # old gathers for the next level: their sources are at levels
            # <= l-1, i.e. rows below L_off[l], so they overlap the AllGather
            if l + 1 < NL:
                emit_seeds(l + 1)
                ninfo = sched["levels"][l + 1]
                Hg_old[l + 1] = (emit_gather(ninfo, "old", l),
                                 emit_onehot(ninfo, "old"))

            # MLP head for this level (fills the AllGather latency, and
            # keeps the PE queue ahead of the slow old-chunk gathers)
            for g, w in enumerate(widths):
                emit_mlp(l, g, w, hnew[g])

            # old chunk matmuls for the next level (gathers by now underway)
            if l + 1 < NL:
                hg_o, oh_o = Hg_old[l + 1]
                emit_chunks(l + 1, "old", hg_o, oh_o)

        for pl in (rpool, tpool, qpool, ppool, hpool, opool, gpool, spool,
                   cpool):
            pl.release()

    nc.compile()
    return nc


# ---------------------------------------------------------------------------
# Entry point
# ---------------------------------------------------------------------------

def _run(inputs, trace=False, reps=1):
    from concourse.bass_utils import run_bass_kernel_spmd

    fl = np.asarray(inputs["forward_level"])
    num_levels = int(fl.max()) + 1
    sched = _preprocess(fl, inputs["edge_index"], num_levels)
    wmat, iota512, vcols_f, vcols_b, vrow_b, pred0 = _prep_weights(inputs)

    key = (sched["N"], sched["TC"], sched["sumVc"], reps,
           tuple(int(x) for x in sched["Vc"]),
           tuple((len(i["old_chunks"]), len(i["fresh_chunks"]))
                 for i in sched["levels"]))
    if key not in _COMPILED:
        _COMPILED[key] = _build(sched, reps=reps)
    nc = _COMPILED[key]

    import ml_dtypes
    bf16 = ml_dtypes.bfloat16
    in_maps = []
    for c in range(NC):
        in_maps.append({
            "wmat": wmat, "iota512": iota512, "vcolsf": vcols_f,
            "vcolsb": vcols_b, "vrowb": vrow_b,
            "n0row": sched["n0row"][c][None, :].astype(bf16),
            "degrow": sched["degrow"][c][None, :].astype(bf16),
            "idxs": sched["idxs"][c],
            "ranks": sched["ranks"][c],
        })

    res = run_bass_kernel_spmd(nc, in_maps, core_ids=list(range(NC)),
                               trace=trace)

    NL = sched["NL"]
    L_off, Vc, Voff = sched["L_off"], sched["Vc"], sched["Voff"]
    node_of_rank = sched["node_of_rank"]
    out = np.zeros(sched["N"], np.float32)
    for c in range(NC):
        oc = res.results[c]["pred"]
        for l in range(NL):
            gr = int(L_off[l]) + c * int(Vc[l]) + np.arange(int(Vc[l]))
            nd = node_of_rank[gr]
            m = nd >= 0
            out[nd[m]] = oc[int(Voff[l]):int(Voff[l]) + int(Vc[l])][m]
    # level-0 nodes all share MLP(h0), computed on host
    out[np.asarray(fl) == 0] = pred0
    return out[:, None], res


def kernel(**inputs):
    out, _ = _run(inputs, trace=False)
    return out


# revision 16
# speedup vs baseline: 2.0239x; 1.0617x over previous
"""DeepSAT GNN message-passing kernel for 8 Trainium2 NeuronCores.

Algorithm notes (validated numerically against the reference):
  - Every node is updated exactly once, at step l = forward_level (levels
    1..19; level-0 nodes keep h0 forever). At update time the node's own
    hidden state is still h0, so the GRU "hidden side" gates are constant
    vectors computable on the host.
  - msg_i = W @ (S_i + n0_i*h0) + deg_i*b, where S_i sums h over "active"
    in-edges (source level in [1, level_i)), n0_i counts inactive in-edges
    and deg_i all in-edges. With u = W^-1 b this folds to
    msg_i = W @ S'_i,  S'_i = S_i + n0_i*h0 + deg_i*u, so the per-gate
    input is  gi_g = (wih_g @ W) @ S'_i + bih_g  -- one fused matmul.
  - Nodes are stored level-sorted ("rank" order): per-level writes are
    contiguous, per-level ownership is an even 8-way split, and the
    AllGather of each level's new h lands in place.
  - The whole h pipeline runs in bf16 (h_store, gathers, one-hot segment
    sums, GRU/MLP weights); PSUM accumulation stays fp32. Level-0 output
    (MLP of the constant h0) is computed on the host.

Device schedule per level l (SPMD on 8 cores):
  gather h[src] for this level's "fresh" chunks (src level == l-1),
  segment-sum via one-hot matmuls into PSUM (seeded with the n0/deg
  terms), fused GRU, PE-transpose, one staged DMA, AllGather into the
  replicated h_store; then (overlapping the collective) the next level's
  seeds + "old" chunks (src level <= l-1, reading rows below this
  level's slab) and this level's MLP head.
"""

import os
import sys
import numpy as np

sys.path.insert(0, "/opt/trn_rl_repo")

P = 128
D = 128
NC = 8
GW = 512  # psum group width (one bank of fp32)

_COMPILED = {}


# ---------------------------------------------------------------------------
# Host-side preprocessing
# ---------------------------------------------------------------------------

def _preprocess(forward_level, edge_index, num_levels):
    fl = np.asarray(forward_level).astype(np.int64)
    ei = np.asarray(edge_index).astype(np.int64)
    src, dst = ei[0], ei[1]
    N = fl.shape[0]
    NL = num_levels

    # --- rank space: nodes sorted by level, each level padded to NC*P ---
    n_l = np.bincount(fl, minlength=NL).astype(np.int64)
    pad_l = ((n_l + NC * P - 1) // (NC * P)) * (NC * P)
    pad_l = np.maximum(pad_l, NC * P)  # at least one block per core
    L_off = np.zeros(NL + 1, np.int64)
    L_off[1:] = np.cumsum(pad_l)
    Vc = (pad_l // NC).astype(np.int64)          # per-core nodes per level
    Voff = np.zeros(NL + 1, np.int64)
    Voff[1:] = np.cumsum(Vc)                     # per-core rank-space offsets
    nblk = (Vc // P).astype(np.int64)

    order = np.argsort(fl, kind="stable")
    starts_real = np.zeros(NL + 1, np.int64)
    starts_real[1:] = np.cumsum(n_l)
    pos_within = np.arange(N, dtype=np.int64) - starts_real[fl[order]]
    rank = np.empty(N, np.int64)
    rank[order] = L_off[fl[order]] + pos_within

    node_of_rank = np.full(L_off[NL], -1, np.int64)
    node_of_rank[rank] = np.arange(N, dtype=np.int64)

    # --- per-node degree stats, indexed by rank ---
    lv_s, lv_d = fl[src], fl[dst]
    act = (lv_s >= 1) & (lv_s < lv_d)
    deg = np.bincount(dst, minlength=N).astype(np.float64)
    n0 = np.bincount(dst[~act], minlength=N).astype(np.float64)

    sumVc = int(Voff[NL])
    n0row = np.zeros((NC, sumVc), np.float32)
    degrow = np.zeros((NC, sumVc), np.float32)
    for c in range(NC):
        # global ranks owned by core c, in per-core rank-space order
        grs = []
        for l in range(NL):
            grs.append(L_off[l] + c * Vc[l] + np.arange(Vc[l]))
        gr = np.concatenate(grs)
        nd = node_of_rank[gr]
        m = nd >= 0
        n0row[c, m] = n0[nd[m]]
        degrow[c, m] = deg[nd[m]]

    # --- active edge table ---
    er = np.where(act)[0]
    e_lvl = lv_d[er]
    e_srcrank = rank[src[er]].astype(np.int64)
    e_dstrank = rank[dst[er]].astype(np.int64)
    e_local = e_dstrank - L_off[e_lvl]
    e_core = e_local // Vc[e_lvl]
    e_wl = e_local % Vc[e_lvl]          # slot within the core's level range
    e_grp = e_wl // GW                  # psum group (512-wide)
    e_fresh = lv_s[er] == (e_lvl - 1)

    # Group edges per (level, phase). Per core the edges are sorted by dst
    # slot and packed into full 128-lane GATHER chunks (one indirect DMA
    # each); gather-chunk counts are the max over cores so the SPMD program
    # is identical everywhere. Each MATMUL chunk is a (gather-chunk, psum
    # group) pair taken from the union over cores: its one-hot spans the
    # group width, with -1 ranks (all-zero one-hot) for lanes of other
    # groups / cores with fewer edges.
    levels = []
    gcol = 0   # gather-chunk columns (idx table)
    mcol = 0   # matmul-chunk columns (rank table)
    idx_cols = [[] for _ in range(NC)]   # each entry: [128] int32 src ranks
    rank_cols = [[] for _ in range(NC)]  # each entry: [128] f32 local slot
    for l in range(NL):
        ngrp = (int(Vc[l]) + GW - 1) // GW
        info = {
            "old_chunks": [],    # (grp, gcol, mcol)
            "fresh_chunks": [],
            "old_g": (gcol, 0),
            "fresh_g": (gcol, 0),
            "old_m": (mcol, 0),
            "fresh_m": (mcol, 0),
            "ngrp": ngrp,
        }
        if l >= 1:
            in_lvl = e_lvl == l
            for phase, sel0 in (("old", in_lvl & ~e_fresh),
                                ("fresh", in_lvl & e_fresh)):
                g_start, m_start = gcol, mcol
                chunks = []
                percore = [np.where(sel0 & (e_core == c))[0] for c in range(NC)]
                percore = [es[np.argsort(e_wl[es], kind="stable")]
                           for es in percore]
                nch = (max(len(es) for es in percore) + P - 1) // P
                for ch in range(nch):
                    lanes = [es[ch * P:(ch + 1) * P] for es in percore]
                    iv = np.zeros((NC, P), np.int32)
                    for c in range(NC):
                        iv[c, : len(lanes[c])] = e_srcrank[lanes[c]]
                        idx_cols[c].append(iv[c])
                    groups = sorted(set(
                        int(g) for c in range(NC) for g in e_grp[lanes[c]]))
                    for g in groups:
                        for c in range(NC):
                            rv = np.full(P, -1.0, np.float32)
                            es = lanes[c]
                            m = e_grp[es] == g
                            rv[: len(es)][m] = e_wl[es[m]] - g * GW
                            rank_cols[c].append(rv)
                        chunks.append((g, gcol, mcol))
                        mcol += 1
                    gcol += 1
                info[phase + "_chunks"] = chunks
                info[phase + "_g"] = (g_start, gcol - g_start)
                info[phase + "_m"] = (m_start, mcol - m_start)
        levels.append(info)

    TCg = max(gcol, 1)
    TCm = max(mcol, 1)
    idxs = np.zeros((NC, P, TCg), np.int32)
    ranks = np.full((NC, P, TCm), -1.0, np.float32)
    for c in range(NC):
        if idx_cols[c]:
            idxs[c, :, :gcol] = np.stack(idx_cols[c], axis=1)
            ranks[c, :, :mcol] = np.stack(rank_cols[c], axis=1)

    # stop-flag bookkeeping: last matmul per (level, grp)
    for l in range(NL):
        info = levels[l]
        last = {}
        for name in ("old_chunks", "fresh_chunks"):
            for (grp, g0, m0) in info[name]:
                last[grp] = (name, m0)
        info["last"] = last

    return {
        "N": N, "NL": NL, "n_l": n_l, "pad": pad_l, "L_off": L_off,
        "Vc": Vc, "Voff": Voff, "nblk": nblk, "sumVc": sumVc,
        "TCg": TCg, "TCm": TCm,
        "levels": levels, "idxs": idxs, "ranks": ranks,
        "n0row": n0row, "degrow": degrow, "node_of_rank": node_of_rank,
    }


def _prep_weights(inp):
    import ml_dtypes
    bf16 = ml_dtypes.bfloat16
    f64 = np.float64
    W = inp["aggr_w"].astype(f64)
    b = inp["aggr_b"].astype(f64)
    h0 = (inp["emd_w"][:, 0] + inp["emd_b"]).astype(f64)
    wih = inp["gru_wih"].astype(f64)
    whh = inp["gru_whh"].astype(f64)
    bih = inp["gru_bih"].astype(f64)
    bhh = inp["gru_bhh"].astype(f64)
    u = np.linalg.solve(W, b)
    assert np.abs(W @ u - b).max() < 1e-6
    ghc = whh @ h0 + bhh
    hr_c, hz_c, hn_c = ghc[:D], ghc[D:2 * D], ghc[2 * D:]
    bih_r, bih_z, bih_n = bih[:D], bih[D:2 * D], bih[2 * D:]
    WgT = [(wih[g * D:(g + 1) * D] @ W).T for g in range(3)]

    W1 = inp["w1"].astype(f64)  # [256, 128]
    W2 = inp["w2"].astype(f64)  # [256, 256]
    w3 = inp["w3"].astype(f64)  # [1, 256]
    DM = W1.shape[0]
    assert DM == 256

    blocks = [
        WgT[0], WgT[1], WgT[2], np.diag(hn_c),
        W1[0:128, :].T, W1[128:256, :].T,
        W2[0:128, 0:128].T, W2[0:128, 128:256].T,
        W2[128:256, 0:128].T, W2[128:256, 128:256].T,
        np.eye(128),
    ]
    wmat = np.concatenate(blocks, axis=1).astype(bf16)  # [128, 11*128] bf16

    iota512 = np.tile(np.arange(GW, dtype=np.float32)[None, :], (P, 1))

    vcols = np.stack([
        h0,                      # 0: h0 column
        bih_r + hr_c,            # 1: sigmoid bias for r
        -(bih_z + hz_c),         # 2: sigmoid bias for z' (scale = -1)
        bih_n,                   # 3: tanh bias for n
        inp["b1"].astype(f64)[0:128],    # 4
        inp["b1"].astype(f64)[128:256],  # 5
        inp["b2"].astype(f64)[0:128],    # 6
        inp["b2"].astype(f64)[128:256],  # 7
        w3[0, 0:128],            # 8
        w3[0, 128:256],          # 9
        np.full(128, inp["b3"].astype(f64)[0]),  # 10: b3 (row 0 used)
    ], axis=1)
    vcols_f = vcols.astype(np.float32)   # [128, 11]
    vcols_b = vcols.astype(bf16)         # [128, 11]

    vrow = np.zeros((1, 256), np.float32)
    vrow[0, :128] = h0.astype(np.float32)
    vrow[0, 128:] = u.astype(np.float32)
    vrow_b = vrow.astype(bf16)

    # level-0 output: MLP(h0), identical for every level-0 node
    z1 = np.maximum(W1 @ h0 + inp["b1"].astype(f64), 0.0)
    z2 = np.maximum(W2 @ z1 + inp["b2"].astype(f64), 0.0)
    pred0 = float(w3[0] @ z2 + inp["b3"].astype(f64)[0])

    return wmat, iota512, vcols_f, vcols_b, vrow_b, pred0


# ---------------------------------------------------------------------------
# Bass program
# ---------------------------------------------------------------------------

WM = {name: i for i, name in enumerate(
    ["WgT_r", "WgT_z", "WgT_n", "diag_hn", "W1Ta", "W1Tb",
     "W2_k0m0", "W2_k1m0", "W2_k0m1", "W2_k1m1", "ident"])}
VC = {name: i for i, name in enumerate(
    ["h0", "bias_r", "nbias_z", "bias_n", "b1a", "b1b", "b2a", "b2b",
     "w3a", "w3b", "b3"])}


def _build(sched, reps=1):
    import concourse.bacc as bacc
    import concourse.tile as tile
    from concourse import bass, mybir

    f32 = mybir.dt.float32
    bf16 = mybir.dt.bfloat16
    i32 = mybir.dt.int32
    AF = mybir.ActivationFunctionType
    OP = mybir.AluOpType
    NL = sched["NL"]
    L_off = sched["L_off"]
    Vc = sched["Vc"]
    Voff = sched["Voff"]
    pad = sched["pad"]
    TCg = sched["TCg"]
    TCm = sched["TCm"]
    sumVc = sched["sumVc"]
    Vmax = int(Vc.max())
    NpadTot = int(L_off[NL])
    RG = [list(range(NC))]

    nc = bacc.Bacc("TRN2", target_bir_lowering=False, debug=False,
                   enable_asserts=False, num_devices=NC)

    wmat_d = nc.dram_tensor("wmat", [P, P * len(WM)], bf16, kind="ExternalInput")
    iota_d = nc.dram_tensor("iota512", [P, GW], f32, kind="ExternalInput")
    vcf_d = nc.dram_tensor("vcolsf", [P, len(VC)], f32, kind="ExternalInput")
    vcb_d = nc.dram_tensor("vcolsb", [P, len(VC)], bf16, kind="ExternalInput")
    vrow_d = nc.dram_tensor("vrowb", [1, 256], bf16, kind="ExternalInput")
    n0_d = nc.dram_tensor("n0row", [1, sumVc], bf16, kind="ExternalInput")
    deg_d = nc.dram_tensor("degrow", [1, sumVc], bf16, kind="ExternalInput")
    idx_d = nc.dram_tensor("idxs", [P, TCg], i32, kind="ExternalInput")
    rnk_d = nc.dram_tensor("ranks", [P, TCm], f32, kind="ExternalInput")
    pred_d = nc.dram_tensor("pred", [sumVc], f32, kind="ExternalOutput")
    h_store = nc.dram_tensor("h_store", [NpadTot, D], bf16, kind="Internal",
                             addr_space="Shared")
    ag_in = [nc.dram_tensor(f"ag_in{i}", [Vmax, D], bf16, kind="Internal")
             for i in range(2)]

    with tile.TileContext(nc) as tc:
        cpool = tc.alloc_tile_pool(name="const", bufs=1)
        spool = tc.alloc_tile_pool(name="sbuf", bufs=2)
        gpool = tc.alloc_tile_pool(name="gath", bufs=2)
        opool = tc.alloc_tile_pool(name="oneh", bufs=24)
        hpool = tc.alloc_tile_pool(name="hnew", bufs=6)
        ppool = tc.alloc_tile_pool(name="psS", bufs=3, space="PSUM")
        qpool = tc.alloc_tile_pool(name="psG", bufs=3, space="PSUM")
        tpool = tc.alloc_tile_pool(name="psT", bufs=1, space="PSUM")
        rpool = tc.alloc_tile_pool(name="psP", bufs=1, space="PSUM")

        # ---- load constants ----
        wm = cpool.tile([P, P * len(WM)], bf16, tag="wm")
        nc.sync.dma_start(out=wm[:], in_=wmat_d[:])
        iota = cpool.tile([P, GW], f32, tag="iota")
        nc.sync.dma_start(out=iota[:], in_=iota_d[:])
        vcf = cpool.tile([P, len(VC)], f32, tag="vcf")
        nc.sync.dma_start(out=vcf[:], in_=vcf_d[:])
        vcb = cpool.tile([P, len(VC)], bf16, tag="vcb")
        nc.sync.dma_start(out=vcb[:], in_=vcb_d[:])
        vr = cpool.tile([1, 256], bf16, tag="vr")
        nc.sync.dma_start(out=vr[:], in_=vrow_d[:])
        idxs = cpool.tile([P, TCg], i32, tag="idxs")
        nc.sync.dma_start(out=idxs[:], in_=idx_d[:])
        rnks = cpool.tile([P, TCm], f32, tag="rnks")
        nc.sync.dma_start(out=rnks[:], in_=rnk_d[:])

        def wmb(name):
            return wm[:, WM[name] * P:(WM[name] + 1) * P]

        def vf(name):
            return vcf[:, VC[name]:VC[name] + 1]

        def vb(name):
            return vcb[:, VC[name]:VC[name] + 1]

        # zero h_store row 0: padded edge lanes gather it, and while their
        # all-zero one-hot column discards the value, 0 * NaN = NaN if the
        # row were left as uninitialized DRAM
        zrow = cpool.tile([1, D], bf16, tag="zrow")
        nc.vector.memset(zrow[:], 0.0)
        zrow_dma = nc.sync.dma_start(out=h_store[0:1, :], in_=zrow[:])

        # ---- per-level state ----
        S_ps = [None] * NL         # list of psum tiles per level (by grp)
        Hg_old = [None] * (NL + 1)
        ccs = {}                   # level -> AllGather instruction

        def grp_widths(l):
            ws = []
            v = int(Vc[l])
            while v > 0:
                ws.append(min(GW, v))
                v -= GW
            return ws

        def emit_onehot(info, which):
            """One [128, GW] bf16 one-hot per MATMUL chunk (fp32 compare)."""
            chunks = info[which + "_chunks"]
            if not chunks:
                return None
            ohs = []
            for (grp, gcol, mcol) in chunks:
                oh = opool.tile([P, GW], bf16, tag="oh" + which)
                nc.vector.tensor_tensor(
                    out=oh[:],
                    in0=rnks[:, mcol:mcol + 1].to_broadcast([P, GW]),
                    in1=iota[:],
                    op=OP.is_equal,
                )
                ohs.append(oh)
            return ohs

        def emit_gather(info, which, bound_level):
            """Gather h rows for one phase; reads only h_store[0:L_off[bound]].

            The prefix bound keeps the read region disjoint from AllGathers
            of levels >= bound_level, so Tile lets them overlap.
            """
            g0, k = info[which + "_g"]
            if k == 0:
                return None
            dep_cc = ccs.get(bound_level - 1)
            hg = gpool.tile([P, k * D], bf16, tag="hg_" + which)
            # the HW indirect DMA consumes ONE index per partition (the
            # interpreter's multi-column semantics do not match silicon), so
            # issue one gather per 128-edge chunk. Padded lanes read row 0;
            # their all-zero one-hot column discards the value.
            for j in range(k):
                gi = nc.gpsimd.indirect_dma_start(
                    out=hg[:, j * D:(j + 1) * D],
                    out_offset=None,
                    in_=h_store[0:int(L_off[bound_level]), :],
                    in_offset=bass.IndirectOffsetOnAxis(
                        ap=idxs[:, g0 + j:g0 + j + 1], axis=0),
                )
                # the dynamic read of h_store is not region-tracked by Tile's
                # shadow memory: pin the RAW edges by hand
                if dep_cc is not None:
                    tile.add_dep_helper(gi.ins, dep_cc.ins, sync=True,
                                        reason="gather reads AllGather output")
                tile.add_dep_helper(gi.ins, zrow_dma.ins, sync=True,
                                    reason="gather may read zeroed row 0")
            return hg

        def emit_seeds(l):
            """allocate S psums for level l and seed with n0*h0 + deg*u."""
            tiles = []
            info = sched["levels"][l]
            v = int(Vc[l])
            off = int(Voff[l])
            n0r = spool.tile([1, Vmax], bf16, tag="n0r")
            nc.sync.dma_start(out=n0r[0:1, :v], in_=n0_d[0:1, off:off + v])
            degr = spool.tile([1, Vmax], bf16, tag="degr")
            nc.sync.dma_start(out=degr[0:1, :v], in_=deg_d[0:1, off:off + v])
            for g, w in enumerate(grp_widths(l)):
                sp = ppool.tile([P, GW], f32, tag="S", space="PSUM")
                nc.tensor.matmul(
                    out=sp[:, :w], lhsT=vr[0:1, 0:128],
                    rhs=n0r[0:1, g * GW:g * GW + w],
                    start=True, stop=False, skip_group_check=True)
                is_last = info["last"].get(g) is None
                nc.tensor.matmul(
                    out=sp[:, :w], lhsT=vr[0:1, 128:256],
                    rhs=degr[0:1, g * GW:g * GW + w],
                    start=False, stop=is_last, skip_group_check=True)
                tiles.append(sp)
            S_ps[l] = tiles

        def emit_chunks(l, which, hg, ohs):
            info = sched["levels"][l]
            chunks = info[which + "_chunks"]
            if not chunks:
                return
            g0 = info[which + "_g"][0]
            widths = grp_widths(l)
            for i, (grp, gcol, mcol) in enumerate(chunks):
                j = gcol - g0
                w = widths[grp]
                is_last = info["last"].get(grp) == (which + "_chunks", mcol)
                nc.tensor.matmul(
                    out=S_ps[l][grp][:, :w],
                    lhsT=hg[:, j * D:(j + 1) * D],
                    rhs=ohs[i][:, :w],
                    start=False, stop=is_last, skip_group_check=True)

        def emit_mlp(l, g, w, rhs_sb):
            """MLP head for one 512-group; writes pred rows."""
            z1s = []
            for half in ("a", "b"):
                zp = qpool.tile([P, GW], f32, tag="G", space="PSUM")
                nc.tensor.matmul(out=zp[:, :w], lhsT=wmb("W1T" + half),
                                 rhs=rhs_sb[:, :w], start=True, stop=True)
                zs = spool.tile([P, GW], bf16, tag="z1" + half)
                nc.vector.tensor_scalar(out=zs[:, :w], in0=zp[:, :w],
                                        scalar1=vf("b1" + half), scalar2=0.0,
                                        op0=OP.add, op1=OP.max)
                z1s.append(zs)
            z2s = []
            for mi, mh in enumerate(("m0", "m1")):
                zp = qpool.tile([P, GW], f32, tag="G", space="PSUM")
                nc.tensor.matmul(out=zp[:, :w], lhsT=wmb("W2_k0" + mh),
                                 rhs=z1s[0][:, :w], start=True, stop=False)
                nc.tensor.matmul(out=zp[:, :w], lhsT=wmb("W2_k1" + mh),
                                 rhs=z1s[1][:, :w], start=False, stop=True)
                zs = spool.tile([P, GW], bf16, tag="z2" + mh)
                nc.vector.tensor_scalar(out=zs[:, :w], in0=zp[:, :w],
                                        scalar1=vf("b2" + ("a" if mi == 0 else "b")),
                                        scalar2=0.0, op0=OP.add, op1=OP.max)
                z2s.append(zs)
            pp = rpool.tile([1, GW], f32, tag="pred", space="PSUM")
            nc.tensor.matmul(out=pp[:, :w], lhsT=vb("w3a"), rhs=z2s[0][:, :w],
                             start=True, stop=False)
            nc.tensor.matmul(out=pp[:, :w], lhsT=vb("w3b"), rhs=z2s[1][:, :w],
                             start=False, stop=True)
            ps = spool.tile([1, GW], f32, tag="psb")
            nc.scalar.activation(out=ps[:, :w], in_=pp[:, :w], func=AF.Identity,
                                 bias=vcf[0:1, VC["b3"]:VC["b3"] + 1])
            off = int(Voff[l]) + g * GW
            nc.sync.dma_start(out=pred_d[off:off + w], in_=ps[0:1, :w])

        def emit_gru(l, g, w):
            """GRU for one 512-group; returns the h_new tile."""
            ssb = spool.tile([P, GW], bf16, tag="Ssb")
            nc.vector.tensor_copy(out=ssb[:, :w], in_=S_ps[l][g][:, :w])

            gr = qpool.tile([P, GW], f32, tag="G", space="PSUM")
            nc.tensor.matmul(out=gr[:, :w], lhsT=wmb("WgT_r"),
                             rhs=ssb[:, :w], start=True, stop=True)
            gz = qpool.tile([P, GW], f32, tag="G", space="PSUM")
            nc.tensor.matmul(out=gz[:, :w], lhsT=wmb("WgT_z"),
                             rhs=ssb[:, :w], start=True, stop=True)
            gn = qpool.tile([P, GW], f32, tag="G", space="PSUM")
            nc.tensor.matmul(out=gn[:, :w], lhsT=wmb("WgT_n"),
                             rhs=ssb[:, :w], start=True, stop=False)

            rsb = spool.tile([P, GW], bf16, tag="rsb")
            nc.scalar.activation(out=rsb[:, :w], in_=gr[:, :w],
                                 func=AF.Sigmoid, bias=vf("bias_r"))
            zsb = spool.tile([P, GW], bf16, tag="zsb")
            nc.scalar.activation(out=zsb[:, :w], in_=gz[:, :w],
                                 func=AF.Sigmoid, bias=vf("nbias_z"),
                                 scale=-1.0)
            nc.tensor.matmul(out=gn[:, :w], lhsT=wmb("diag_hn"),
                             rhs=rsb[:, :w], start=False, stop=True)
            nsb = spool.tile([P, GW], bf16, tag="nsb")
            nc.scalar.activation(out=nsb[:, :w], in_=gn[:, :w],
                                 func=AF.Tanh, bias=vf("bias_n"))

            t3 = spool.tile([P, GW], bf16, tag="t3")
            nc.vector.tensor_scalar(out=t3[:, :w], in0=nsb[:, :w],
                                    scalar1=vf("h0"), scalar2=None,
                                    op0=OP.subtract)
            t4 = spool.tile([P, GW], bf16, tag="t4")
            nc.vector.tensor_tensor(out=t4[:, :w], in0=t3[:, :w],
                                    in1=zsb[:, :w], op=OP.mult)
            hn = hpool.tile([P, GW], bf16, tag="hnew")
            nc.vector.tensor_scalar(out=hn[:, :w], in0=t4[:, :w],
                                    scalar1=vf("h0"), scalar2=None,
                                    op0=OP.add)
            return hn

        # reps>1 repeats the whole computation for wall-clock timing: the
        # computation is idempotent (h_store/pred rewritten with same values)
        for _rep in range(reps):
          # seeds + (no chunks possible) for level 1
          emit_seeds(1)
          Oh_fresh = emit_onehot(sched["levels"][1], "fresh")

          # ================= levels 1..NL-1 =================
          for l in range(1, NL):
            info = sched["levels"][l]
            widths = grp_widths(l)
            ngrp = len(widths)
            v = int(Vc[l])

            # fresh gather + chunks for this level (critical path; the
            # one-hots were prebuilt during the previous level)
            hg_f = emit_gather(info, "fresh", l)
            emit_chunks(l, "fresh", hg_f, Oh_fresh)

            # GRU group 0 first, then overlap group 1's matmuls with group
            # 0's activation chain before transposing group 0 - the staged
            # h feeds the AllGather as early as possible
            hnew = [None] * ngrp
            hnew[0] = emit_gru(l, 0, widths[0])
            if ngrp > 1:
                hnew[1] = emit_gru(l, 1, widths[1])

            hT = None
            if l < NL - 1:
                hT = spool.tile([P, Vmax], bf16, tag="hT")

                def transp(g):
                    w = widths[g]
                    tp = tpool.tile([P, GW], bf16, tag="tp", space="PSUM")
                    for b in range(w // P):
                        nc.tensor.transpose(
                            out=tp[:, b * P:(b + 1) * P],
                            in_=hnew[g][:, b * P:(b + 1) * P],
                            identity=wmb("ident"))
                    nc.vector.tensor_copy(out=hT[:, g * GW:g * GW + w],
                                          in_=tp[:, :w])

                transp(0)
            for g in range(2, ngrp):
                hnew[g] = emit_gru(l, g, widths[g])
            if l < NL - 1:
                for g in range(1, ngrp):
                    transp(g)
                agt = ag_in[l % 2]
                nc.sync.dma_start(
                    out=agt[0:v, :].rearrange("(blk p) f -> p blk f", p=P),
                    in_=hT[:, 0:v].rearrange("p (blk f) -> p blk f", f=P))
                cc = nc.gpsimd.collective_compute(
                    "AllGather", bass.mybir.AluOpType.bypass,
                    replica_groups=RG,
                    ins=[agt[0:v, :].opt()],
                    outs=[h_store[int(L_off[l]):int(L_off[l]) + int(pad[l]), :].opt()],
                )
                ccs[l] = cc

            # seeds for the next level (cheap, keeps PSUM group order), then
            # old gathers for the next level: their sources are at levels
            # <= l-1, i.e. rows below L_off[l], so they overlap the AllGather.
            # The next level's fresh one-hots are also prebuilt here so they
            # are off the critical path.
            if l + 1 < NL:
                emit_seeds(l + 1)
                ninfo = sched["levels"][l + 1]
                Hg_old[l + 1] = (emit_gather(ninfo, "old", l),
                                 emit_onehot(ninfo, "old"))
                Oh_fresh = emit_onehot(ninfo, "fresh")

            # MLP head for this level (fills the AllGather latency, and
            # keeps the PE queue ahead of the slow old-chunk gathers)
            for g, w in enumerate(widths):
                emit_mlp(l, g, w, hnew[g])

            # old chunk matmuls for the next level (gathers by now underway)
            if l + 1 < NL:
                hg_o, oh_o = Hg_old[l + 1]
                emit_chunks(l + 1, "old", hg_o, oh_o)

        for pl in (rpool, tpool, qpool, ppool, hpool, opool, gpool, spool,
                   cpool):
            pl.release()

    nc.compile()
    return nc


# ---------------------------------------------------------------------------
# Entry point
# ---------------------------------------------------------------------------

def _run(inputs, trace=False, reps=1):
    from concourse.bass_utils import run_bass_kernel_spmd

    fl = np.asarray(inputs["forward_level"])
    num_levels = int(fl.max()) + 1
    sched = _preprocess(fl, inputs["edge_index"], num_levels)
    wmat, iota512, vcols_f, vcols_b, vrow_b, pred0 = _prep_weights(inputs)

    key = (sched["N"], sched["TCg"], sched["TCm"], sched["sumVc"], reps,
           tuple(int(x) for x in sched["Vc"]),
           tuple((len(i["old_chunks"]), len(i["fresh_chunks"]))
                 for i in sched["levels"]))
    if key not in _COMPILED:
        _COMPILED[key] = _build(sched, reps=reps)
    nc = _COMPILED[key]

    import ml_dtypes
    bf16 = ml_dtypes.bfloat16
    in_maps = []
    for c in range(NC):
        in_maps.append({
            "wmat": wmat, "iota512": iota512, "vcolsf": vcols_f,
            "vcolsb": vcols_b, "vrowb": vrow_b,
            "n0row": sched["n0row"][c][None, :].astype(bf16),
            "degrow": sched["degrow"][c][None, :].astype(bf16),
            "idxs": sched["idxs"][c],
            "ranks": sched["ranks"][c],
        })

    res = run_bass_kernel_spmd(nc, in_maps, core_ids=list(range(NC)),
                               trace=trace)

    NL = sched["NL"]
    L_off, Vc, Voff = sched["L_off"], sched["Vc"], sched["Voff"]
    node_of_rank = sched["node_of_rank"]
    out = np.zeros(sched["N"], np.float32)
    for c in range(NC):
        oc = res.results[c]["pred"]
        for l in range(NL):
            gr = int(L_off[l]) + c * int(Vc[l]) + np.arange(int(Vc[l]))
            nd = node_of_rank[gr]
            m = nd >= 0
            out[nd[m]] = oc[int(Voff[l]):int(Voff[l]) + int(Vc[l])][m]
    # level-0 nodes all share MLP(h0), computed on host
    out[np.asarray(fl) == 0] = pred0
    return out[:, None], res


def kernel(**inputs):
    out, _ = _run(inputs, trace=False)
    return out


# revision 17
# speedup vs baseline: 2.3101x; 1.1414x over previous
"""DeepSAT GNN message-passing kernel for 8 Trainium2 NeuronCores.

Algorithm notes (validated numerically against the reference):
  - Every node is updated exactly once, at step l = forward_level (levels
    1..19; level-0 nodes keep h0 forever, and their MLP output is a single
    host-computed constant). At update time the node's own hidden state is
    still h0, so the GRU "hidden side" gates are constant vectors
    computable on the host.
  - msg_i = W @ (S_i + n0_i*h0) + deg_i*b, where S_i sums h over "active"
    in-edges (source level in [1, level_i)), n0_i counts inactive in-edges
    and deg_i all in-edges. With u = W^-1 b this folds to
    msg_i = W @ S'_i,  S'_i = S_i + n0_i*h0 + deg_i*u, so the per-gate
    input is  gi_g = (wih_g @ W) @ S'_i + bih_g  -- one fused matmul.
  - Per level, each core's nodes are ordered [sources of next-level
    "fresh" edges | sources of deeper "old" edges | never-consumed rows].
    Only the first two regions are exchanged: a small "fresh" AllGather
    (on the critical path, dispatched right after GRU group 0) and a
    "bulk" AllGather with a full level of slack. Never-consumed rows stay
    local. The whole h pipeline runs in bf16; PSUM stays fp32.
  - Gathers of h[src] split into three phases by source level: "fresh"
    (src = dst_level-1, needs the latest fresh AllGather), "shal"
    (src = dst_level-2, issued one level early), "deep" (src <=
    dst_level-3, issued two levels early) - this spreads the serial SWDGE
    cost of indirect DMAs across the pipeline.
"""

import os
import sys
import numpy as np

sys.path.insert(0, "/opt/trn_rl_repo")

P = 128
D = 128
NC = 8
GW = 512  # psum group width (one bank of fp32)

_COMPILED = {}

PHASES = ("shal", "deep", "fresh")  # "last"-priority order: fresh wins


# ---------------------------------------------------------------------------
# Host-side preprocessing
# ---------------------------------------------------------------------------

def _preprocess(forward_level, edge_index, num_levels):
    fl = np.asarray(forward_level).astype(np.int64)
    ei = np.asarray(edge_index).astype(np.int64)
    src, dst = ei[0], ei[1]
    N = fl.shape[0]
    NL = num_levels

    lv_s, lv_d = fl[src], fl[dst]
    act = (lv_s >= 1) & (lv_s < lv_d)
    ae = np.where(act)[0]
    fresh_needed = np.zeros(N, bool)
    old_needed = np.zeros(N, bool)
    fe = lv_d[ae] == lv_s[ae] + 1
    fresh_needed[src[ae[fe]]] = True
    old_needed[src[ae[~fe]]] = True

    deg = np.bincount(dst, minlength=N).astype(np.float64)
    n0 = np.bincount(dst[~act], minlength=N).astype(np.float64)

    # --- per level: deal nodes to cores round-robin per category, order
    # [fresh | old | rest] within each core, pad regions to K/B/Vc ---
    K = np.zeros(NL, np.int64)
    B = np.zeros(NL, np.int64)
    Vc = np.zeros(NL, np.int64)
    hoff = np.zeros(NL + 1, np.int64)
    hoff[1] = 1  # slab row 0 is the zeroed dummy row for padded edge lanes
    core_nodes = [[None] * NL for _ in range(NC)]  # per (core, level): node ids

    def rup(x, m):
        return ((x + m - 1) // m) * m

    for l in range(NL):
        nodes_l = np.where(fl == l)[0]
        if l == 0:
            # level 0 never runs on the device and is never a source
            Vc[l] = 0
            hoff[1] = 1
            for c in range(NC):
                core_nodes[c][l] = np.empty(0, np.int64)
            continue
        fr = nodes_l[fresh_needed[nodes_l]]
        on = nodes_l[old_needed[nodes_l] & ~fresh_needed[nodes_l]]
        rs = nodes_l[~fresh_needed[nodes_l] & ~old_needed[nodes_l]]
        Kl = rup(max(len(fr[c::NC]) for c in range(NC)), P) if len(fr) else 0
        Bl = rup(max(len(on[c::NC]) for c in range(NC)), P) if len(on) else 0
        assert Kl <= GW, f"fresh region {Kl} exceeds one psum group"
        vc = rup(Kl + Bl + max(len(rs[c::NC]) for c in range(NC)), P)
        vc = max(vc, P)
        K[l], B[l], Vc[l] = Kl, Bl, vc
        for c in range(NC):
            core_nodes[c][l] = (fr[c::NC], on[c::NC], rs[c::NC])

    for l in range(1, NL):
        hoff[l + 1] = hoff[l] + NC * (K[l] + B[l])
    SLAB = int(hoff[NL])

    Voff = np.zeros(NL + 1, np.int64)
    Voff[1:] = np.cumsum(Vc)
    sumVc = int(Voff[NL])

    # node -> (core, pos-within-core-level) and slab index
    node_pos = np.full(N, -1, np.int64)
    node_core = np.full(N, -1, np.int64)
    srcslab = np.full(N, -1, np.int64)
    nodes_arr = np.full((NC, sumVc), -1, np.int64)
    n0row = np.zeros((NC, sumVc), np.float32)
    degrow = np.zeros((NC, sumVc), np.float32)
    for l in range(1, NL):
        for c in range(NC):
            fr, on, rs = core_nodes[c][l]
            pos = np.concatenate([
                np.arange(len(fr)),
                int(K[l]) + np.arange(len(on)),
                int(K[l] + B[l]) + np.arange(len(rs)),
            ])
            nd = np.concatenate([fr, on, rs])
            node_pos[nd] = pos
            node_core[nd] = c
            slab = np.where(
                pos < K[l], hoff[l] + c * K[l] + pos,
                hoff[l] + NC * K[l] + c * B[l] + (pos - K[l]))
            srcslab[nd[pos < K[l] + B[l]]] = slab[pos < K[l] + B[l]]
            nodes_arr[c, Voff[l] + pos] = nd
            n0row[c, Voff[l] + pos] = n0[nd]
            degrow[c, Voff[l] + pos] = deg[nd]

    # --- active edge table, split by phase ---
    er = ae
    e_lvl = lv_d[er]
    e_src = src[er]
    e_dst = dst[er]
    assert (srcslab[e_src] >= 0).all()
    e_srcslab = srcslab[e_src]
    e_core = node_core[e_dst]
    e_wl = node_pos[e_dst]
    e_grp = e_wl // GW
    gap = e_lvl - lv_s[er]  # 1 = fresh, 2 = shal, >=3 = deep

    levels = []
    gcol = 0
    mcol = 0
    idx_cols = [[] for _ in range(NC)]
    rank_cols = [[] for _ in range(NC)]
    for l in range(NL):
        ngrp = (int(Vc[l]) + GW - 1) // GW
        info = {"ngrp": ngrp}
        for ph in PHASES:
            info[ph + "_chunks"] = []
            info[ph + "_g"] = (gcol, 0)
        if l >= 1:
            in_lvl = e_lvl == l
            for ph, sel0 in (("shal", in_lvl & (gap == 2)),
                             ("deep", in_lvl & (gap >= 3)),
                             ("fresh", in_lvl & (gap == 1))):
                g_start = gcol
                chunks = []
                percore = [np.where(sel0 & (e_core == c))[0] for c in range(NC)]
                percore = [es[np.argsort(e_wl[es], kind="stable")]
                           for es in percore]
                nch = (max(len(es) for es in percore) + P - 1) // P
                for ch in range(nch):
                    lanes = [es[ch * P:(ch + 1) * P] for es in percore]
                    for c in range(NC):
                        iv = np.zeros(P, np.int32)
                        iv[: len(lanes[c])] = e_srcslab[lanes[c]]
                        idx_cols[c].append(iv)
                    groups = sorted(set(
                        int(g) for c in range(NC) for g in e_grp[lanes[c]]))
                    for g in groups:
                        for c in range(NC):
                            rv = np.full(P, -1.0, np.float16)
                            es = lanes[c]
                            m = e_grp[es] == g
                            rv[: len(es)][m] = (e_wl[es[m]] - g * GW).astype(
                                np.float16)
                            rank_cols[c].append(rv)
                        chunks.append((g, gcol, mcol))
                        mcol += 1
                    gcol += 1
                info[ph + "_chunks"] = chunks
                info[ph + "_g"] = (g_start, gcol - g_start)
        levels.append(info)

    TCg = max(gcol, 1)
    TCm = max(mcol, 1)
    idxs = np.zeros((NC, P, TCg), np.int32)
    ranks = np.full((NC, P, TCm), -1.0, np.float16)
    for c in range(NC):
        if idx_cols[c]:
            idxs[c, :, :gcol] = np.stack(idx_cols[c], axis=1)
            ranks[c, :, :mcol] = np.stack(rank_cols[c], axis=1)

    # stop-flag bookkeeping: last matmul per (level, grp); PHASES order
    # matches device emission order (shal, deep, then fresh at level start)
    for l in range(NL):
        info = levels[l]
        last = {}
        for ph in PHASES:
            for (grp, g0, m0) in info[ph + "_chunks"]:
                last[grp] = (ph, m0)
        info["last"] = last

    return {
        "N": N, "NL": NL, "K": K, "B": B, "Vc": Vc, "Voff": Voff,
        "hoff": hoff, "SLAB": SLAB, "sumVc": sumVc,
        "TCg": TCg, "TCm": TCm,
        "levels": levels, "idxs": idxs, "ranks": ranks,
        "n0row": n0row, "degrow": degrow, "nodes_arr": nodes_arr,
        "fl": fl,
    }


def _prep_weights(inp):
    import ml_dtypes
    bf16 = ml_dtypes.bfloat16
    f64 = np.float64
    W = inp["aggr_w"].astype(f64)
    b = inp["aggr_b"].astype(f64)
    h0 = (inp["emd_w"][:, 0] + inp["emd_b"]).astype(f64)
    wih = inp["gru_wih"].astype(f64)
    whh = inp["gru_whh"].astype(f64)
    bih = inp["gru_bih"].astype(f64)
    bhh = inp["gru_bhh"].astype(f64)
    u = np.linalg.solve(W, b)
    assert np.abs(W @ u - b).max() < 1e-6
    ghc = whh @ h0 + bhh
    hr_c, hz_c, hn_c = ghc[:D], ghc[D:2 * D], ghc[2 * D:]
    bih_r, bih_z, bih_n = bih[:D], bih[D:2 * D], bih[2 * D:]
    WgT = [(wih[g * D:(g + 1) * D] @ W).T for g in range(3)]

    W1 = inp["w1"].astype(f64)  # [256, 128]
    W2 = inp["w2"].astype(f64)  # [256, 256]
    w3 = inp["w3"].astype(f64)  # [1, 256]
    assert W1.shape[0] == 256

    blocks = [
        WgT[0], WgT[1], WgT[2], np.diag(hn_c),
        W1[0:128, :].T, W1[128:256, :].T,
        W2[0:128, 0:128].T, W2[0:128, 128:256].T,
        W2[128:256, 0:128].T, W2[128:256, 128:256].T,
        np.eye(128),
    ]
    wmat = np.concatenate(blocks, axis=1).astype(bf16)  # [128, 11*128] bf16

    iota512 = np.tile(np.arange(GW, dtype=np.float16)[None, :], (P, 1))

    vcols = np.stack([
        h0,                      # 0: h0 column
        -h0,                     # 1: -h0 (scalar-engine bias for t3)
        bih_r + hr_c,            # 2: sigmoid bias for r
        -(bih_z + hz_c),         # 3: sigmoid bias for z' (scale = -1)
        bih_n,                   # 4: tanh bias for n
        inp["b1"].astype(f64)[0:128],    # 5
        inp["b1"].astype(f64)[128:256],  # 6
        inp["b2"].astype(f64)[0:128],    # 7
        inp["b2"].astype(f64)[128:256],  # 8
        w3[0, 0:128],            # 9
        w3[0, 128:256],          # 10
        np.full(128, inp["b3"].astype(f64)[0]),  # 11: b3 (row 0 used)
    ], axis=1)
    vcols_f = vcols.astype(np.float32)   # [128, 12]
    vcols_b = vcols.astype(bf16)         # [128, 12]

    vr2 = np.stack([h0, u], axis=0).astype(bf16)  # [2, 128]

    # level-0 output: MLP(h0), identical for every level-0 node
    z1 = np.maximum(W1 @ h0 + inp["b1"].astype(f64), 0.0)
    z2 = np.maximum(W2 @ z1 + inp["b2"].astype(f64), 0.0)
    pred0 = float(w3[0] @ z2 + inp["b3"].astype(f64)[0])

    return wmat, iota512, vcols_f, vcols_b, vr2, pred0


# ---------------------------------------------------------------------------
# Bass program
# ---------------------------------------------------------------------------

WM = {name: i for i, name in enumerate(
    ["WgT_r", "WgT_z", "WgT_n", "diag_hn", "W1Ta", "W1Tb",
     "W2_k0m0", "W2_k1m0", "W2_k0m1", "W2_k1m1", "ident"])}
VC = {name: i for i, name in enumerate(
    ["h0", "nh0", "bias_r", "nbias_z", "bias_n", "b1a", "b1b", "b2a", "b2b",
     "w3a", "w3b", "b3"])}


def _build(sched, reps=1):
    import concourse.bacc as bacc
    import concourse.tile as tile
    from concourse import bass, mybir

    f32 = mybir.dt.float32
    f16 = mybir.dt.float16
    bf16 = mybir.dt.bfloat16
    i32 = mybir.dt.int32
    AF = mybir.ActivationFunctionType
    OP = mybir.AluOpType
    NL = sched["NL"]
    K = sched["K"]
    B = sched["B"]
    Vc = sched["Vc"]
    Voff = sched["Voff"]
    hoff = sched["hoff"]
    SLAB = sched["SLAB"]
    TCg = sched["TCg"]
    TCm = sched["TCm"]
    sumVc = sched["sumVc"]
    Vmax = int(Vc.max())
    KBmax = int((K + B).max())
    RG = [list(range(NC))]

    nc = bacc.Bacc("TRN2", target_bir_lowering=False, debug=False,
                   enable_asserts=False, num_devices=NC)

    wmat_d = nc.dram_tensor("wmat", [P, P * len(WM)], bf16, kind="ExternalInput")
    iota_d = nc.dram_tensor("iota512", [P, GW], f16, kind="ExternalInput")
    vcf_d = nc.dram_tensor("vcolsf", [P, len(VC)], f32, kind="ExternalInput")
    vcb_d = nc.dram_tensor("vcolsb", [P, len(VC)], bf16, kind="ExternalInput")
    vr2_d = nc.dram_tensor("vr2", [2, P], bf16, kind="ExternalInput")
    nd_d = nc.dram_tensor("n0deg", [2, sumVc], bf16, kind="ExternalInput")
    idx_d = nc.dram_tensor("idxs", [P, TCg], i32, kind="ExternalInput")
    rnk_d = nc.dram_tensor("ranks", [P, TCm], f16, kind="ExternalInput")
    pred_d = nc.dram_tensor("pred", [sumVc], f32, kind="ExternalOutput")
    h_slab = nc.dram_tensor("h_slab", [SLAB, D], bf16, kind="Internal",
                            addr_space="Shared")
    ag_in = [nc.dram_tensor(f"ag_in{i}", [max(KBmax, P), D], bf16,
                            kind="Internal") for i in range(2)]

    with tile.TileContext(nc) as tc:
        cpool = tc.alloc_tile_pool(name="const", bufs=1)
        spool = tc.alloc_tile_pool(name="sbuf", bufs=2)
        gpool = tc.alloc_tile_pool(name="gath", bufs=2)
        opool = tc.alloc_tile_pool(name="oneh", bufs=24)
        hpool = tc.alloc_tile_pool(name="hnew", bufs=6)
        ppool = tc.alloc_tile_pool(name="psS", bufs=3, space="PSUM")
        qpool = tc.alloc_tile_pool(name="psG", bufs=3, space="PSUM")
        tpool = tc.alloc_tile_pool(name="psT", bufs=1, space="PSUM")
        rpool = tc.alloc_tile_pool(name="psP", bufs=1, space="PSUM")

        # ---- load constants ----
        wm = cpool.tile([P, P * len(WM)], bf16, tag="wm")
        nc.sync.dma_start(out=wm[:], in_=wmat_d[:])
        iota = cpool.tile([P, GW], f16, tag="iota")
        nc.sync.dma_start(out=iota[:], in_=iota_d[:])
        vcf = cpool.tile([P, len(VC)], f32, tag="vcf")
        nc.sync.dma_start(out=vcf[:], in_=vcf_d[:])
        vcb = cpool.tile([P, len(VC)], bf16, tag="vcb")
        nc.sync.dma_start(out=vcb[:], in_=vcb_d[:])
        vr2 = cpool.tile([2, P], bf16, tag="vr2")
        nc.sync.dma_start(out=vr2[:], in_=vr2_d[:])
        idxs = cpool.tile([P, TCg], i32, tag="idxs")
        nc.sync.dma_start(out=idxs[:], in_=idx_d[:])
        rnks = cpool.tile([P, TCm], f16, tag="rnks")
        nc.sync.dma_start(out=rnks[:], in_=rnk_d[:])

        def wmb(name):
            return wm[:, WM[name] * P:(WM[name] + 1) * P]

        def vf(name):
            return vcf[:, VC[name]:VC[name] + 1]

        def vb(name):
            return vcb[:, VC[name]:VC[name] + 1]

        # zero slab row 0: padded edge lanes gather it, and while their
        # all-zero one-hot column discards the value, 0 * NaN = NaN if the
        # row were left as uninitialized DRAM
        zrow = cpool.tile([1, D], bf16, tag="zrow")
        nc.vector.memset(zrow[:], 0.0)
        zrow_dma = nc.sync.dma_start(out=h_slab[0:1, :], in_=zrow[:])

        # ---- per-level state ----
        S_ps = [None] * NL
        ccF = {}   # level -> fresh AllGather
        ccB = {}   # level -> bulk AllGather

        def grp_widths(l):
            ws = []
            v = int(Vc[l])
            while v > 0:
                ws.append(min(GW, v))
                v -= GW
            return ws

        def emit_onehot(info, ph):
            chunks = info[ph + "_chunks"]
            if not chunks:
                return None
            ohs = []
            for (grp, gcol, mcol) in chunks:
                oh = opool.tile([P, GW], bf16, tag="oh" + ph)
                nc.vector.tensor_tensor(
                    out=oh[:],
                    in0=rnks[:, mcol:mcol + 1].to_broadcast([P, GW]),
                    in1=iota[:],
                    op=OP.is_equal,
                )
                ohs.append(oh)
            return ohs

        def emit_gather(info, ph, bound, deps):
            """Gather h rows for one phase; reads only h_slab[0:bound]."""
            g0, k = info[ph + "_g"]
            if k == 0:
                return None
            hg = gpool.tile([P, k * D], bf16, tag="hg_" + ph)
            for j in range(k):
                gi = nc.gpsimd.indirect_dma_start(
                    out=hg[:, j * D:(j + 1) * D],
                    out_offset=None,
                    in_=h_slab[0:bound, :],
                    in_offset=bass.IndirectOffsetOnAxis(
                        ap=idxs[:, g0 + j:g0 + j + 1], axis=0),
                )
                # dynamic reads of h_slab are not region-tracked by Tile's
                # shadow memory: pin the RAW edges by hand
                for dep in deps:
                    if dep is not None:
                        tile.add_dep_helper(gi.ins, dep.ins, sync=True,
                                            reason="gather reads AllGather out")
                tile.add_dep_helper(gi.ins, zrow_dma.ins, sync=True,
                                    reason="gather may read zeroed row 0")
            return hg

        def emit_seeds(l):
            """allocate S psums for level l, seed with h0 (x) n0 + u (x) deg."""
            tiles = []
            info = sched["levels"][l]
            v = int(Vc[l])
            off = int(Voff[l])
            ndr = spool.tile([2, Vmax], bf16, tag="ndr")
            nc.sync.dma_start(out=ndr[:, :v], in_=nd_d[:, off:off + v])
            for g, w in enumerate(grp_widths(l)):
                sp = ppool.tile([P, GW], f32, tag="S", space="PSUM")
                is_last = info["last"].get(g) is None
                nc.tensor.matmul(
                    out=sp[:, :w], lhsT=vr2[:],
                    rhs=ndr[:, g * GW:g * GW + w],
                    start=True, stop=is_last, skip_group_check=True)
                tiles.append(sp)
            S_ps[l] = tiles

        def emit_chunks(l, ph, hg, ohs):
            info = sched["levels"][l]
            chunks = info[ph + "_chunks"]
            if not chunks:
                return
            g0 = info[ph + "_g"][0]
            widths = grp_widths(l)
            for i, (grp, gcol, mcol) in enumerate(chunks):
                j = gcol - g0
                w = widths[grp]
                is_last = info["last"].get(grp) == (ph, mcol)
                nc.tensor.matmul(
                    out=S_ps[l][grp][:, :w],
                    lhsT=hg[:, j * D:(j + 1) * D],
                    rhs=ohs[i][:, :w],
                    start=False, stop=is_last, skip_group_check=True)

        def emit_mlp(l, g, w, rhs_sb):
            """MLP head for one 512-group; writes pred rows."""
            z1s = []
            for half in ("a", "b"):
                zp = qpool.tile([P, GW], f32, tag="G", space="PSUM")
                nc.tensor.matmul(out=zp[:, :w], lhsT=wmb("W1T" + half),
                                 rhs=rhs_sb[:, :w], start=True, stop=True)
                zs = spool.tile([P, GW], bf16, tag="z1" + half)
                nc.scalar.activation(out=zs[:, :w], in_=zp[:, :w],
                                     func=AF.Relu, bias=vf("b1" + half))
                z1s.append(zs)
            z2s = []
            for mi, mh in enumerate(("m0", "m1")):
                zp = qpool.tile([P, GW], f32, tag="G", space="PSUM")
                nc.tensor.matmul(out=zp[:, :w], lhsT=wmb("W2_k0" + mh),
                                 rhs=z1s[0][:, :w], start=True, stop=False)
                nc.tensor.matmul(out=zp[:, :w], lhsT=wmb("W2_k1" + mh),
                                 rhs=z1s[1][:, :w], start=False, stop=True)
                zs = spool.tile([P, GW], bf16, tag="z2" + mh)
                nc.vector.tensor_scalar(out=zs[:, :w], in0=zp[:, :w],
                                        scalar1=vf("b2" + ("a" if mi == 0 else "b")),
                                        scalar2=0.0, op0=OP.add, op1=OP.max)
                z2s.append(zs)
            pp = rpool.tile([1, GW], f32, tag="pred", space="PSUM")
            nc.tensor.matmul(out=pp[:, :w], lhsT=vb("w3a"), rhs=z2s[0][:, :w],
                             start=True, stop=False)
            nc.tensor.matmul(out=pp[:, :w], lhsT=vb("w3b"), rhs=z2s[1][:, :w],
                             start=False, stop=True)
            ps = spool.tile([1, GW], f32, tag="psb")
            nc.scalar.activation(out=ps[:, :w], in_=pp[:, :w], func=AF.Identity,
                                 bias=vcf[0:1, VC["b3"]:VC["b3"] + 1])
            off = int(Voff[l]) + g * GW
            nc.sync.dma_start(out=pred_d[off:off + w], in_=ps[0:1, :w])

        def emit_gru(l, g, w):
            """GRU for one 512-group; returns the h_new tile."""
            ssb = spool.tile([P, GW], bf16, tag="Ssb")
            nc.vector.tensor_copy(out=ssb[:, :w], in_=S_ps[l][g][:, :w])

            gr = qpool.tile([P, GW], f32, tag="G", space="PSUM")
            nc.tensor.matmul(out=gr[:, :w], lhsT=wmb("WgT_r"),
                             rhs=ssb[:, :w], start=True, stop=True)
            gz = qpool.tile([P, GW], f32, tag="G", space="PSUM")
            nc.tensor.matmul(out=gz[:, :w], lhsT=wmb("WgT_z"),
                             rhs=ssb[:, :w], start=True, stop=True)
            gn = qpool.tile([P, GW], f32, tag="G", space="PSUM")
            nc.tensor.matmul(out=gn[:, :w], lhsT=wmb("WgT_n"),
                             rhs=ssb[:, :w], start=True, stop=False)

            rsb = spool.tile([P, GW], bf16, tag="rsb")
            nc.scalar.activation(out=rsb[:, :w], in_=gr[:, :w],
                                 func=AF.Sigmoid, bias=vf("bias_r"))
            zsb = spool.tile([P, GW], bf16, tag="zsb")
            nc.scalar.activation(out=zsb[:, :w], in_=gz[:, :w],
                                 func=AF.Sigmoid, bias=vf("nbias_z"),
                                 scale=-1.0)
            nc.tensor.matmul(out=gn[:, :w], lhsT=wmb("diag_hn"),
                             rhs=rsb[:, :w], start=False, stop=True)
            nsb = spool.tile([P, GW], bf16, tag="nsb")
            nc.scalar.activation(out=nsb[:, :w], in_=gn[:, :w],
                                 func=AF.Tanh, bias=vf("bias_n"))

            # t3 = n - h0 on the scalar engine (keeps DVE free)
            t3 = spool.tile([P, GW], bf16, tag="t3")
            nc.scalar.activation(out=t3[:, :w], in_=nsb[:, :w],
                                 func=AF.Identity, bias=vf("nh0"))
            t4 = spool.tile([P, GW], bf16, tag="t4")
            nc.vector.tensor_tensor(out=t4[:, :w], in0=t3[:, :w],
                                    in1=zsb[:, :w], op=OP.mult)
            hn = hpool.tile([P, GW], bf16, tag="hnew")
            nc.scalar.activation(out=hn[:, :w], in_=t4[:, :w],
                                 func=AF.Identity, bias=vf("h0"))
            return hn

        # reps>1 repeats the whole computation for wall-clock timing: the
        # computation is idempotent (h_slab/pred rewritten with same values)
        for _rep in range(reps):
          emit_seeds(1)
          Oh = {ph: None for ph in PHASES}
          Hg = {ph: None for ph in PHASES}  # most recent gather per phase
          Hg_deep_next = None               # deep gather issued 2 levels early

          # ================= levels 1..NL-1 =================
          for l in range(1, NL):
            info = sched["levels"][l]
            widths = grp_widths(l)
            ngrp = len(widths)
            Kl, Bl = int(K[l]), int(B[l])
            KB = Kl + Bl

            # fresh gather + chunks for this level (critical path; one-hots
            # prebuilt during the previous level)
            hg_f = emit_gather(info, "fresh",
                               int(hoff[l - 1] + NC * K[l - 1]) if l >= 2 else 1,
                               [ccF.get(l - 1)])
            emit_chunks(l, "fresh", hg_f, Oh["fresh"])

            # GRU group 0 and 1; then the fresh region (first K rows, inside
            # group 0) is transposed, staged and AllGathered immediately
            hnew = [None] * ngrp
            hnew[0] = emit_gru(l, 0, widths[0])
            if ngrp > 1:
                hnew[1] = emit_gru(l, 1, widths[1])

            hT = None
            agt = ag_in[l % 2]

            def transp_blocks(lo_blk, hi_blk):
                cur_g = None
                tp = None
                lo_b = None
                hi_b = None

                def flush():
                    nc.vector.tensor_copy(
                        out=hT[:, cur_g * GW + lo_b * P:cur_g * GW + (hi_b + 1) * P],
                        in_=tp[:, lo_b * P:(hi_b + 1) * P])

                for blk in range(lo_blk, hi_blk):
                    g, bb = blk // 4, blk % 4
                    if g != cur_g:
                        if cur_g is not None:
                            flush()
                        tp = tpool.tile([P, GW], bf16, tag="tp", space="PSUM")
                        cur_g, lo_b = g, bb
                    hi_b = bb
                    nc.tensor.transpose(
                        out=tp[:, bb * P:(bb + 1) * P],
                        in_=hnew[g][:, bb * P:(bb + 1) * P],
                        identity=wmb("ident"))
                if cur_g is not None:
                    flush()

            if l < NL - 1 and KB > 0:
                hT = spool.tile([P, KBmax], bf16, tag="hT")
                if Kl > 0:
                    transp_blocks(0, Kl // P)
                    nc.sync.dma_start(
                        out=agt[0:Kl, :].rearrange("(blk p) f -> p blk f", p=P),
                        in_=hT[:, 0:Kl].rearrange("p (blk f) -> p blk f", f=P))
                    cc = nc.gpsimd.collective_compute(
                        "AllGather", bass.mybir.AluOpType.bypass,
                        replica_groups=RG,
                        ins=[agt[0:Kl, :].opt()],
                        outs=[h_slab[int(hoff[l]):int(hoff[l]) + NC * Kl, :].opt()],
                    )
                    ccF[l] = cc

            for g in range(2, ngrp):
                hnew[g] = emit_gru(l, g, widths[g])

            # shallow gathers for l+1 (src level l-1) and deep gathers for
            # l+2 (src <= l-1): both read rows below hoff[l], written by the
            # l-1 AllGathers, so they run during this level's collectives
            if l + 1 < NL:
                ninfo = sched["levels"][l + 1]
                Hg["shal"] = emit_gather(ninfo, "shal", int(hoff[l]),
                                         [ccF.get(l - 1), ccB.get(l - 1)])
            if l + 2 < NL:
                nninfo = sched["levels"][l + 2]
                Hg_deep_next = emit_gather(nninfo, "deep", int(hoff[l]),
                                           [ccF.get(l - 1), ccB.get(l - 1)])

            # bulk region: transpose, stage, AllGather (a full level of slack)
            if l < NL - 1 and Bl > 0:
                transp_blocks(Kl // P, KB // P)
                nc.sync.dma_start(
                    out=agt[Kl:KB, :].rearrange("(blk p) f -> p blk f", p=P),
                    in_=hT[:, Kl:KB].rearrange("p (blk f) -> p blk f", f=P))
                cc = nc.gpsimd.collective_compute(
                    "AllGather", bass.mybir.AluOpType.bypass,
                    replica_groups=RG,
                    ins=[agt[Kl:KB, :].opt()],
                    outs=[h_slab[int(hoff[l]) + NC * Kl:int(hoff[l]) + NC * KB,
                                 :].opt()],
                )
                ccB[l] = cc

            # next level's seeds + MLP head for this level (overlap work)
            if l + 1 < NL:
                emit_seeds(l + 1)
            for g, w in enumerate(widths):
                emit_mlp(l, g, w, hnew[g])

            # next level's shal/deep matmuls + prebuild of fresh one-hots
            if l + 1 < NL:
                ninfo = sched["levels"][l + 1]
                oh_s = emit_onehot(ninfo, "shal")
                emit_chunks(l + 1, "shal", Hg["shal"], oh_s)
                oh_d = emit_onehot(ninfo, "deep")
                emit_chunks(l + 1, "deep", Hg["deep"], oh_d)
                Oh["fresh"] = emit_onehot(ninfo, "fresh")
            Hg["deep"] = Hg_deep_next

        for pl in (rpool, tpool, qpool, ppool, hpool, opool, gpool, spool,
                   cpool):
            pl.release()

    nc.compile()
    return nc


# ---------------------------------------------------------------------------
# Entry point
# ---------------------------------------------------------------------------

def _run(inputs, trace=False, reps=1):
    from concourse.bass_utils import run_bass_kernel_spmd

    fl = np.asarray(inputs["forward_level"])
    num_levels = int(fl.max()) + 1
    sched = _preprocess(fl, inputs["edge_index"], num_levels)
    wmat, iota512, vcols_f, vcols_b, vr2, pred0 = _prep_weights(inputs)

    key = (sched["N"], sched["TCg"], sched["TCm"], sched["sumVc"], reps,
           tuple(int(x) for x in sched["Vc"]),
           tuple(int(x) for x in sched["K"]),
           tuple(int(x) for x in sched["B"]),
           tuple((len(i["shal_chunks"]), len(i["deep_chunks"]),
                  len(i["fresh_chunks"])) for i in sched["levels"]))
    if key not in _COMPILED:
        _COMPILED[key] = _build(sched, reps=reps)
    nc = _COMPILED[key]

    import ml_dtypes
    bf16 = ml_dtypes.bfloat16
    in_maps = []
    for c in range(NC):
        nd = np.stack([sched["n0row"][c], sched["degrow"][c]], axis=0)
        in_maps.append({
            "wmat": wmat, "iota512": iota512, "vcolsf": vcols_f,
            "vcolsb": vcols_b, "vr2": vr2,
            "n0deg": nd.astype(bf16),
            "idxs": sched["idxs"][c],
            "ranks": sched["ranks"][c],
        })

    res = run_bass_kernel_spmd(nc, in_maps, core_ids=list(range(NC)),
                               trace=trace)

    nodes_arr = sched["nodes_arr"]
    out = np.zeros(sched["N"], np.float32)
    for c in range(NC):
        oc = res.results[c]["pred"]
        m = nodes_arr[c] >= 0
        out[nodes_arr[c][m]] = oc[m]
    out[fl == 0] = pred0
    return out[:, None], res


def kernel(**inputs):
    out, _ = _run(inputs, trace=False)
    return out


# revision 21
# speedup vs baseline: 2.4109x; 1.0436x over previous
"""DeepSAT GNN message-passing kernel for 8 Trainium2 NeuronCores.

Algorithm notes (validated numerically against the reference):
  - Every node is updated exactly once, at step l = forward_level (levels
    1..19; level-0 nodes keep h0 forever, and their MLP output is a single
    host-computed constant). At update time the node's own hidden state is
    still h0, so the GRU "hidden side" gates are constant vectors
    computable on the host.
  - msg_i = W @ (S_i + n0_i*h0) + deg_i*b, where S_i sums h over "active"
    in-edges (source level in [1, level_i)), n0_i counts inactive in-edges
    and deg_i all in-edges. With u = W^-1 b this folds to
    msg_i = W @ S'_i,  S'_i = S_i + n0_i*h0 + deg_i*u, so the per-gate
    input is  gi_g = (wih_g @ W) @ S'_i + bih_g  -- one fused matmul.
  - Per level, each core's nodes are ordered [sources of next-level
    "fresh" edges | sources of deeper "old" edges | never-consumed rows].
    Only the first two regions are exchanged: a small "fresh" AllGather
    (on the critical path, dispatched right after GRU group 0) and a
    "bulk" AllGather with a full level of slack. Never-consumed rows stay
    local. The whole h pipeline runs in bf16; PSUM stays fp32.
  - Gathers of h[src] split into three phases by source level: "fresh"
    (src = dst_level-1, needs the latest fresh AllGather), "shal"
    (src = dst_level-2, issued one level early), "deep" (src <=
    dst_level-3, issued two levels early) - this spreads the serial SWDGE
    cost of indirect DMAs across the pipeline.
"""

import os
import sys
import numpy as np

sys.path.insert(0, "/opt/trn_rl_repo")

P = 128
D = 128
NC = 8
GW = 512  # psum group width (one bank of fp32)

_COMPILED = {}

PHASES = ("shal", "deep", "fresh")  # "last"-priority order: fresh wins


# ---------------------------------------------------------------------------
# Host-side preprocessing
# ---------------------------------------------------------------------------

def _preprocess(forward_level, edge_index, num_levels):
    fl = np.asarray(forward_level).astype(np.int64)
    ei = np.asarray(edge_index).astype(np.int64)
    src, dst = ei[0], ei[1]
    N = fl.shape[0]
    NL = num_levels

    lv_s, lv_d = fl[src], fl[dst]
    act = (lv_s >= 1) & (lv_s < lv_d)
    ae = np.where(act)[0]
    fresh_needed = np.zeros(N, bool)
    old_needed = np.zeros(N, bool)
    fe = lv_d[ae] == lv_s[ae] + 1
    fresh_needed[src[ae[fe]]] = True
    old_needed[src[ae[~fe]]] = True

    deg = np.bincount(dst, minlength=N).astype(np.float64)
    n0 = np.bincount(dst[~act], minlength=N).astype(np.float64)

    # --- per level: deal nodes to cores round-robin per category, order
    # [fresh | old | rest] within each core, pad regions to K/B/Vc ---
    K = np.zeros(NL, np.int64)
    B = np.zeros(NL, np.int64)
    Vc = np.zeros(NL, np.int64)
    hoff = np.zeros(NL + 1, np.int64)
    hoff[1] = 1  # slab row 0 is the zeroed dummy row for padded edge lanes
    core_nodes = [[None] * NL for _ in range(NC)]  # per (core, level): node ids

    def rup(x, m):
        return ((x + m - 1) // m) * m

    for l in range(NL):
        nodes_l = np.where(fl == l)[0]
        if l == 0:
            # level 0 never runs on the device and is never a source
            Vc[l] = 0
            hoff[1] = 1
            for c in range(NC):
                core_nodes[c][l] = np.empty(0, np.int64)
            continue
        fr = nodes_l[fresh_needed[nodes_l]]
        on = nodes_l[old_needed[nodes_l] & ~fresh_needed[nodes_l]]
        rs = nodes_l[~fresh_needed[nodes_l] & ~old_needed[nodes_l]]
        Kl = rup(max(len(fr[c::NC]) for c in range(NC)), P) if len(fr) else 0
        Bl = rup(max(len(on[c::NC]) for c in range(NC)), P) if len(on) else 0
        assert Kl <= GW, f"fresh region {Kl} exceeds one psum group"
        vc = rup(Kl + Bl + max(len(rs[c::NC]) for c in range(NC)), P)
        vc = max(vc, P)
        K[l], B[l], Vc[l] = Kl, Bl, vc
        for c in range(NC):
            core_nodes[c][l] = (fr[c::NC], on[c::NC], rs[c::NC])

    for l in range(1, NL):
        hoff[l + 1] = hoff[l] + NC * (K[l] + B[l])
    SLAB = int(hoff[NL])

    Voff = np.zeros(NL + 1, np.int64)
    Voff[1:] = np.cumsum(Vc)
    sumVc = int(Voff[NL])

    # node -> (core, pos-within-core-level) and slab index
    node_pos = np.full(N, -1, np.int64)
    node_core = np.full(N, -1, np.int64)
    srcslab = np.full(N, -1, np.int64)
    nodes_arr = np.full((NC, sumVc), -1, np.int64)
    n0row = np.zeros((NC, sumVc), np.float32)
    degrow = np.zeros((NC, sumVc), np.float32)
    for l in range(1, NL):
        for c in range(NC):
            fr, on, rs = core_nodes[c][l]
            pos = np.concatenate([
                np.arange(len(fr)),
                int(K[l]) + np.arange(len(on)),
                int(K[l] + B[l]) + np.arange(len(rs)),
            ])
            nd = np.concatenate([fr, on, rs])
            node_pos[nd] = pos
            node_core[nd] = c
            slab = np.where(
                pos < K[l], hoff[l] + c * K[l] + pos,
                hoff[l] + NC * K[l] + c * B[l] + (pos - K[l]))
            srcslab[nd[pos < K[l] + B[l]]] = slab[pos < K[l] + B[l]]
            nodes_arr[c, Voff[l] + pos] = nd
            n0row[c, Voff[l] + pos] = n0[nd]
            degrow[c, Voff[l] + pos] = deg[nd]

    # --- active edge table, split by phase ---
    er = ae
    e_lvl = lv_d[er]
    e_src = src[er]
    e_dst = dst[er]
    assert (srcslab[e_src] >= 0).all()
    e_srcslab = srcslab[e_src]
    e_core = node_core[e_dst]
    e_wl = node_pos[e_dst]
    e_grp = e_wl // GW
    gap = e_lvl - lv_s[er]  # 1 = fresh, 2 = shal, >=3 = deep

    levels = []
    gcol = 0
    mcol = 0
    idx_cols = [[] for _ in range(NC)]
    rank_cols = [[] for _ in range(NC)]
    for l in range(NL):
        ngrp = (int(Vc[l]) + GW - 1) // GW
        info = {"ngrp": ngrp}
        for ph in PHASES:
            info[ph + "_chunks"] = []
            info[ph + "_g"] = (gcol, 0)
        if l >= 1:
            in_lvl = e_lvl == l
            for ph, sel0 in (("shal", in_lvl & (gap == 2)),
                             ("deep", in_lvl & (gap >= 3)),
                             ("fresh", in_lvl & (gap == 1))):
                g_start = gcol
                chunks = []
                percore = [np.where(sel0 & (e_core == c))[0] for c in range(NC)]
                percore = [es[np.argsort(e_wl[es], kind="stable")]
                           for es in percore]
                nch = (max(len(es) for es in percore) + P - 1) // P
                for ch in range(nch):
                    lanes = [es[ch * P:(ch + 1) * P] for es in percore]
                    for c in range(NC):
                        iv = np.zeros(P, np.int32)
                        iv[: len(lanes[c])] = e_srcslab[lanes[c]]
                        idx_cols[c].append(iv)
                    groups = sorted(set(
                        int(g) for c in range(NC) for g in e_grp[lanes[c]]))
                    for g in groups:
                        # narrow the one-hot / matmul to the slot range this
                        # chunk actually touches within the group (edges are
                        # slot-sorted, so the range is tight)
                        slots = np.concatenate([
                            e_wl[lanes[c]][e_grp[lanes[c]] == g]
                            for c in range(NC)])
                        lo = int(slots.min()) - g * GW
                        wd = min(int(slots.max()) - g * GW - lo + 1, GW)
                        for c in range(NC):
                            rv = np.full(P, -1.0, np.float16)
                            es = lanes[c]
                            m = e_grp[es] == g
                            rv[: len(es)][m] = (e_wl[es[m]] - g * GW - lo
                                                ).astype(np.float16)
                            rank_cols[c].append(rv)
                        chunks.append((g, lo, wd, gcol, mcol))
                        mcol += 1
                    gcol += 1
                info[ph + "_chunks"] = chunks
                info[ph + "_g"] = (g_start, gcol - g_start)
        levels.append(info)

    TCg = max(gcol, 1)
    TCm = max(mcol, 1)
    idxs = np.zeros((NC, P, TCg), np.int32)
    ranks = np.full((NC, P, TCm), -1.0, np.float16)
    for c in range(NC):
        if idx_cols[c]:
            idxs[c, :, :gcol] = np.stack(idx_cols[c], axis=1)
            ranks[c, :, :mcol] = np.stack(rank_cols[c], axis=1)

    # stop-flag bookkeeping: last matmul per (level, grp); PHASES order
    # matches device emission order (shal, deep, then fresh at level start)
    for l in range(NL):
        info = levels[l]
        last = {}
        for ph in PHASES:
            for (grp, lo, wd, g0, m0) in info[ph + "_chunks"]:
                last[grp] = (ph, m0)
        info["last"] = last

    return {
        "N": N, "NL": NL, "K": K, "B": B, "Vc": Vc, "Voff": Voff,
        "hoff": hoff, "SLAB": SLAB, "sumVc": sumVc,
        "TCg": TCg, "TCm": TCm,
        "levels": levels, "idxs": idxs, "ranks": ranks,
        "n0row": n0row, "degrow": degrow, "nodes_arr": nodes_arr,
        "fl": fl,
    }


def _prep_weights(inp):
    import ml_dtypes
    bf16 = ml_dtypes.bfloat16
    f64 = np.float64
    W = inp["aggr_w"].astype(f64)
    b = inp["aggr_b"].astype(f64)
    h0 = (inp["emd_w"][:, 0] + inp["emd_b"]).astype(f64)
    wih = inp["gru_wih"].astype(f64)
    whh = inp["gru_whh"].astype(f64)
    bih = inp["gru_bih"].astype(f64)
    bhh = inp["gru_bhh"].astype(f64)
    u = np.linalg.solve(W, b)
    assert np.abs(W @ u - b).max() < 1e-6
    ghc = whh @ h0 + bhh
    hr_c, hz_c, hn_c = ghc[:D], ghc[D:2 * D], ghc[2 * D:]
    bih_r, bih_z, bih_n = bih[:D], bih[D:2 * D], bih[2 * D:]
    WgT = [(wih[g * D:(g + 1) * D] @ W).T for g in range(3)]

    W1 = inp["w1"].astype(f64)  # [256, 128]
    W2 = inp["w2"].astype(f64)  # [256, 256]
    w3 = inp["w3"].astype(f64)  # [1, 256]
    assert W1.shape[0] == 256

    blocks = [
        WgT[0], WgT[1], WgT[2], np.diag(hn_c),
        W1[0:128, :].T, W1[128:256, :].T,
        W2[0:128, 0:128].T, W2[0:128, 128:256].T,
        W2[128:256, 0:128].T, W2[128:256, 128:256].T,
        np.eye(128),
    ]
    wmat = np.concatenate(blocks, axis=1).astype(bf16)  # [128, 11*128] bf16

    iota512 = np.tile(np.arange(GW, dtype=np.float16)[None, :], (P, 1))

    vcols = np.stack([
        h0,                      # 0: h0 column
        -h0,                     # 1: -h0 (scalar-engine bias for t3)
        bih_r + hr_c,            # 2: sigmoid bias for r
        -(bih_z + hz_c),         # 3: sigmoid bias for z' (scale = -1)
        bih_n,                   # 4: tanh bias for n
        inp["b1"].astype(f64)[0:128],    # 5
        inp["b1"].astype(f64)[128:256],  # 6
        inp["b2"].astype(f64)[0:128],    # 7
        inp["b2"].astype(f64)[128:256],  # 8
        w3[0, 0:128],            # 9
        w3[0, 128:256],          # 10
        np.full(128, inp["b3"].astype(f64)[0]),  # 11: b3 (row 0 used)
    ], axis=1)
    vcols_f = vcols.astype(np.float32)   # [128, 12]
    vcols_b = vcols.astype(bf16)         # [128, 12]

    vr2 = np.stack([h0, u], axis=0).astype(bf16)  # [2, 128]

    # level-0 output: MLP(h0), identical for every level-0 node
    z1 = np.maximum(W1 @ h0 + inp["b1"].astype(f64), 0.0)
    z2 = np.maximum(W2 @ z1 + inp["b2"].astype(f64), 0.0)
    pred0 = float(w3[0] @ z2 + inp["b3"].astype(f64)[0])

    return wmat, iota512, vcols_f, vcols_b, vr2, pred0


# ---------------------------------------------------------------------------
# Bass program
# ---------------------------------------------------------------------------

WM = {name: i for i, name in enumerate(
    ["WgT_r", "WgT_z", "WgT_n", "diag_hn", "W1Ta", "W1Tb",
     "W2_k0m0", "W2_k1m0", "W2_k0m1", "W2_k1m1", "ident"])}
VC = {name: i for i, name in enumerate(
    ["h0", "nh0", "bias_r", "nbias_z", "bias_n", "b1a", "b1b", "b2a", "b2b",
     "w3a", "w3b", "b3"])}


def _build(sched, reps=1):
    import concourse.bacc as bacc
    import concourse.tile as tile
    from concourse import bass, mybir

    f32 = mybir.dt.float32
    f16 = mybir.dt.float16
    bf16 = mybir.dt.bfloat16
    i32 = mybir.dt.int32
    AF = mybir.ActivationFunctionType
    OP = mybir.AluOpType
    NL = sched["NL"]
    K = sched["K"]
    B = sched["B"]
    Vc = sched["Vc"]
    Voff = sched["Voff"]
    hoff = sched["hoff"]
    SLAB = sched["SLAB"]
    TCg = sched["TCg"]
    TCm = sched["TCm"]
    sumVc = sched["sumVc"]
    Vmax = int(Vc.max())
    KBmax = int((K + B).max())
    RG = [list(range(NC))]

    nc = bacc.Bacc("TRN2", target_bir_lowering=False, debug=False,
                   enable_asserts=False, num_devices=NC)

    wmat_d = nc.dram_tensor("wmat", [P, P * len(WM)], bf16, kind="ExternalInput")
    iota_d = nc.dram_tensor("iota512", [P, GW], f16, kind="ExternalInput")
    vcf_d = nc.dram_tensor("vcolsf", [P, len(VC)], f32, kind="ExternalInput")
    vcb_d = nc.dram_tensor("vcolsb", [P, len(VC)], bf16, kind="ExternalInput")
    vr2_d = nc.dram_tensor("vr2", [2, P], bf16, kind="ExternalInput")
    nd_d = nc.dram_tensor("n0deg", [2, sumVc], bf16, kind="ExternalInput")
    idx_d = nc.dram_tensor("idxs", [P, TCg], i32, kind="ExternalInput")
    rnk_d = nc.dram_tensor("ranks", [P, TCm], f16, kind="ExternalInput")
    pred_d = nc.dram_tensor("pred", [sumVc], f32, kind="ExternalOutput")
    h_slab = nc.dram_tensor("h_slab", [SLAB, D], bf16, kind="Internal",
                            addr_space="Shared")
    ag_in = [nc.dram_tensor(f"ag_in{i}", [max(KBmax, P), D], bf16,
                            kind="Internal") for i in range(2)]

    with tile.TileContext(nc) as tc:
        cpool = tc.alloc_tile_pool(name="const", bufs=1)
        spool = tc.alloc_tile_pool(name="sbuf", bufs=2)
        gpool = tc.alloc_tile_pool(name="gath", bufs=2)
        opool = tc.alloc_tile_pool(name="oneh", bufs=24)
        hpool = tc.alloc_tile_pool(name="hnew", bufs=6)
        ppool = tc.alloc_tile_pool(name="psS", bufs=3, space="PSUM")
        qpool = tc.alloc_tile_pool(name="psG", bufs=3, space="PSUM")
        tpool = tc.alloc_tile_pool(name="psT", bufs=1, space="PSUM")
        rpool = tc.alloc_tile_pool(name="psP", bufs=1, space="PSUM")

        # ---- load constants ----
        wm = cpool.tile([P, P * len(WM)], bf16, tag="wm")
        nc.sync.dma_start(out=wm[:], in_=wmat_d[:])
        iota = cpool.tile([P, GW], f16, tag="iota")
        nc.sync.dma_start(out=iota[:], in_=iota_d[:])
        vcf = cpool.tile([P, len(VC)], f32, tag="vcf")
        nc.sync.dma_start(out=vcf[:], in_=vcf_d[:])
        vcb = cpool.tile([P, len(VC)], bf16, tag="vcb")
        nc.sync.dma_start(out=vcb[:], in_=vcb_d[:])
        vr2 = cpool.tile([2, P], bf16, tag="vr2")
        nc.sync.dma_start(out=vr2[:], in_=vr2_d[:])
        idxs = cpool.tile([P, TCg], i32, tag="idxs")
        nc.sync.dma_start(out=idxs[:], in_=idx_d[:])
        rnks = cpool.tile([P, TCm], f16, tag="rnks")
        nc.sync.dma_start(out=rnks[:], in_=rnk_d[:])

        def wmb(name):
            return wm[:, WM[name] * P:(WM[name] + 1) * P]

        def vf(name):
            return vcf[:, VC[name]:VC[name] + 1]

        def vb(name):
            return vcb[:, VC[name]:VC[name] + 1]

        # zero slab row 0: padded edge lanes gather it, and while their
        # all-zero one-hot column discards the value, 0 * NaN = NaN if the
        # row were left as uninitialized DRAM
        zrow = cpool.tile([1, D], bf16, tag="zrow")
        nc.vector.memset(zrow[:], 0.0)
        zrow_dma = nc.sync.dma_start(out=h_slab[0:1, :], in_=zrow[:])

        # ---- per-level state ----
        S_ps = [None] * NL
        ccF = {}   # level -> fresh AllGather
        ccB = {}   # level -> bulk AllGather

        def grp_widths(l):
            ws = []
            v = int(Vc[l])
            while v > 0:
                ws.append(min(GW, v))
                v -= GW
            return ws

        def emit_onehot(info, ph):
            chunks = info[ph + "_chunks"]
            if not chunks:
                return None
            ohs = []
            for (grp, lo, wd, gcol, mcol) in chunks:
                oh = opool.tile([P, GW], bf16, tag="oh" + ph)
                nc.vector.tensor_tensor(
                    out=oh[:, :wd],
                    in0=rnks[:, mcol:mcol + 1].to_broadcast([P, wd]),
                    in1=iota[:, :wd],
                    op=OP.is_equal,
                )
                ohs.append(oh)
            return ohs

        def emit_gather(info, ph, bound, deps):
            """Gather h rows for one phase; reads only h_slab[0:bound]."""
            g0, k = info[ph + "_g"]
            if k == 0:
                return None
            hg = gpool.tile([P, k * D], bf16, tag="hg_" + ph)
            for j in range(k):
                gi = nc.gpsimd.indirect_dma_start(
                    out=hg[:, j * D:(j + 1) * D],
                    out_offset=None,
                    in_=h_slab[0:bound, :],
                    in_offset=bass.IndirectOffsetOnAxis(
                        ap=idxs[:, g0 + j:g0 + j + 1], axis=0),
                )
                # dynamic reads of h_slab are not region-tracked by Tile's
                # shadow memory: pin the RAW edges by hand
                for dep in deps:
                    if dep is not None:
                        tile.add_dep_helper(gi.ins, dep.ins, sync=True,
                                            reason="gather reads AllGather out")
                tile.add_dep_helper(gi.ins, zrow_dma.ins, sync=True,
                                    reason="gather may read zeroed row 0")
            return hg

        def emit_seeds(l):
            """allocate S psums for level l, seed with h0 (x) n0 + u (x) deg."""
            tiles = []
            info = sched["levels"][l]
            v = int(Vc[l])
            off = int(Voff[l])
            ndr = spool.tile([2, Vmax], bf16, tag="ndr")
            nc.sync.dma_start(out=ndr[:, :v], in_=nd_d[:, off:off + v])
            for g, w in enumerate(grp_widths(l)):
                sp = ppool.tile([P, GW], f32, tag="S", space="PSUM")
                is_last = info["last"].get(g) is None
                nc.tensor.matmul(
                    out=sp[:, :w], lhsT=vr2[:],
                    rhs=ndr[:, g * GW:g * GW + w],
                    start=True, stop=is_last, skip_group_check=True)
                tiles.append(sp)
            S_ps[l] = tiles

        def emit_chunks(l, ph, hg, ohs):
            info = sched["levels"][l]
            chunks = info[ph + "_chunks"]
            if not chunks:
                return
            g0 = info[ph + "_g"][0]
            for i, (grp, lo, wd, gcol, mcol) in enumerate(chunks):
                j = gcol - g0
                is_last = info["last"].get(grp) == (ph, mcol)
                nc.tensor.matmul(
                    out=S_ps[l][grp][:, lo:lo + wd],
                    lhsT=hg[:, j * D:(j + 1) * D],
                    rhs=ohs[i][:, :wd],
                    start=False, stop=is_last, skip_group_check=True)

        def emit_mlp(l, g, w, rhs_sb):
            """MLP head for one 512-group; writes pred rows."""
            z1s = []
            for half in ("a", "b"):
                zp = qpool.tile([P, GW], f32, tag="G", space="PSUM")
                nc.tensor.matmul(out=zp[:, :w], lhsT=wmb("W1T" + half),
                                 rhs=rhs_sb[:, :w], start=True, stop=True)
                zs = spool.tile([P, GW], bf16, tag="z1" + half)
                nc.scalar.activation(out=zs[:, :w], in_=zp[:, :w],
                                     func=AF.Relu, bias=vf("b1" + half))
                z1s.append(zs)
            z2s = []
            for mi, mh in enumerate(("m0", "m1")):
                zp = qpool.tile([P, GW], f32, tag="G", space="PSUM")
                nc.tensor.matmul(out=zp[:, :w], lhsT=wmb("W2_k0" + mh),
                                 rhs=z1s[0][:, :w], start=True, stop=False)
                nc.tensor.matmul(out=zp[:, :w], lhsT=wmb("W2_k1" + mh),
                                 rhs=z1s[1][:, :w], start=False, stop=True)
                zs = spool.tile([P, GW], bf16, tag="z2" + mh)
                nc.vector.tensor_scalar(out=zs[:, :w], in0=zp[:, :w],
                                        scalar1=vf("b2" + ("a" if mi == 0 else "b")),
                                        scalar2=0.0, op0=OP.add, op1=OP.max)
                z2s.append(zs)
            pp = rpool.tile([1, GW], f32, tag="pred", space="PSUM")
            nc.tensor.matmul(out=pp[:, :w], lhsT=vb("w3a"), rhs=z2s[0][:, :w],
                             start=True, stop=False)
            nc.tensor.matmul(out=pp[:, :w], lhsT=vb("w3b"), rhs=z2s[1][:, :w],
                             start=False, stop=True)
            ps = spool.tile([1, GW], f32, tag="psb")
            nc.scalar.activation(out=ps[:, :w], in_=pp[:, :w], func=AF.Identity,
                                 bias=vcf[0:1, VC["b3"]:VC["b3"] + 1])
            off = int(Voff[l]) + g * GW
            nc.sync.dma_start(out=pred_d[off:off + w], in_=ps[0:1, :w])

        def emit_gru(l, g, w):
            """GRU for one 512-group; returns the h_new tile."""
            ssb = spool.tile([P, GW], bf16, tag="Ssb")
            nc.vector.tensor_copy(out=ssb[:, :w], in_=S_ps[l][g][:, :w])

            gr = qpool.tile([P, GW], f32, tag="G", space="PSUM")
            nc.tensor.matmul(out=gr[:, :w], lhsT=wmb("WgT_r"),
                             rhs=ssb[:, :w], start=True, stop=True)
            gz = qpool.tile([P, GW], f32, tag="G", space="PSUM")
            nc.tensor.matmul(out=gz[:, :w], lhsT=wmb("WgT_z"),
                             rhs=ssb[:, :w], start=True, stop=True)
            gn = qpool.tile([P, GW], f32, tag="G", space="PSUM")
            nc.tensor.matmul(out=gn[:, :w], lhsT=wmb("WgT_n"),
                             rhs=ssb[:, :w], start=True, stop=False)

            rsb = spool.tile([P, GW], bf16, tag="rsb")
            nc.scalar.activation(out=rsb[:, :w], in_=gr[:, :w],
                                 func=AF.Sigmoid, bias=vf("bias_r"))
            zsb = spool.tile([P, GW], bf16, tag="zsb")
            nc.scalar.activation(out=zsb[:, :w], in_=gz[:, :w],
                                 func=AF.Sigmoid, bias=vf("nbias_z"),
                                 scale=-1.0)
            nc.tensor.matmul(out=gn[:, :w], lhsT=wmb("diag_hn"),
                             rhs=rsb[:, :w], start=False, stop=True)
            nsb = spool.tile([P, GW], bf16, tag="nsb")
            nc.scalar.activation(out=nsb[:, :w], in_=gn[:, :w],
                                 func=AF.Tanh, bias=vf("bias_n"))

            # t3 = n - h0 on the scalar engine (keeps DVE free)
            t3 = spool.tile([P, GW], bf16, tag="t3")
            nc.scalar.activation(out=t3[:, :w], in_=nsb[:, :w],
                                 func=AF.Identity, bias=vf("nh0"))
            t4 = spool.tile([P, GW], bf16, tag="t4")
            nc.vector.tensor_tensor(out=t4[:, :w], in0=t3[:, :w],
                                    in1=zsb[:, :w], op=OP.mult)
            hn = hpool.tile([P, GW], bf16, tag="hnew")
            nc.scalar.activation(out=hn[:, :w], in_=t4[:, :w],
                                 func=AF.Identity, bias=vf("h0"))
            return hn

        # reps>1 repeats the whole computation for wall-clock timing: the
        # computation is idempotent (h_slab/pred rewritten with same values)
        for _rep in range(reps):
          emit_seeds(1)
          Oh = {ph: None for ph in PHASES}
          Hg = {ph: None for ph in PHASES}  # most recent gather per phase
          Hg_deep_next = None               # deep gather issued 2 levels early

          # ================= levels 1..NL-1 =================
          for l in range(1, NL):
            info = sched["levels"][l]
            widths = grp_widths(l)
            ngrp = len(widths)
            Kl, Bl = int(K[l]), int(B[l])
            KB = Kl + Bl

            # fresh gather + chunks for this level (critical path; one-hots
            # prebuilt during the previous level)
            hg_f = emit_gather(info, "fresh",
                               int(hoff[l - 1] + NC * K[l - 1]) if l >= 2 else 1,
                               [ccF.get(l - 1)])
            emit_chunks(l, "fresh", hg_f, Oh["fresh"])

            # GRU group 0 and 1; then the fresh region (first K rows, inside
            # group 0) is transposed, staged and AllGathered immediately
            hnew = [None] * ngrp
            hnew[0] = emit_gru(l, 0, widths[0])
            if ngrp > 1:
                hnew[1] = emit_gru(l, 1, widths[1])

            hT = None
            agt = ag_in[l % 2]

            def transp_blocks(lo_blk, hi_blk):
                cur_g = None
                tp = None
                lo_b = None
                hi_b = None

                def flush():
                    nc.vector.tensor_copy(
                        out=hT[:, cur_g * GW + lo_b * P:cur_g * GW + (hi_b + 1) * P],
                        in_=tp[:, lo_b * P:(hi_b + 1) * P])

                for blk in range(lo_blk, hi_blk):
                    g, bb = blk // 4, blk % 4
                    if g != cur_g:
                        if cur_g is not None:
                            flush()
                        tp = tpool.tile([P, GW], bf16, tag="tp", space="PSUM")
                        cur_g, lo_b = g, bb
                    hi_b = bb
                    nc.tensor.transpose(
                        out=tp[:, bb * P:(bb + 1) * P],
                        in_=hnew[g][:, bb * P:(bb + 1) * P],
                        identity=wmb("ident"))
                if cur_g is not None:
                    flush()

            if l < NL - 1 and KB > 0:
                hT = spool.tile([P, KBmax], bf16, tag="hT")
                if Kl > 0:
                    transp_blocks(0, Kl // P)
                    nc.sync.dma_start(
                        out=agt[0:Kl, :].rearrange("(blk p) f -> p blk f", p=P),
                        in_=hT[:, 0:Kl].rearrange("p (blk f) -> p blk f", f=P))
                    cc = nc.gpsimd.collective_compute(
                        "AllGather", bass.mybir.AluOpType.bypass,
                        replica_groups=RG,
                        ins=[agt[0:Kl, :].opt()],
                        outs=[h_slab[int(hoff[l]):int(hoff[l]) + NC * Kl, :].opt()],
                    )
                    ccF[l] = cc

            for g in range(2, ngrp):
                hnew[g] = emit_gru(l, g, widths[g])

            # shallow gathers for l+1 (src level l-1) and deep gathers for
            # l+2 (src <= l-1): both read rows below hoff[l], written by the
            # l-1 AllGathers, so they run during this level's collectives
            if l + 1 < NL:
                ninfo = sched["levels"][l + 1]
                Hg["shal"] = emit_gather(ninfo, "shal", int(hoff[l]),
                                         [ccF.get(l - 1), ccB.get(l - 1)])
            if l + 2 < NL:
                nninfo = sched["levels"][l + 2]
                Hg_deep_next = emit_gather(nninfo, "deep", int(hoff[l]),
                                           [ccF.get(l - 1), ccB.get(l - 1)])

            # bulk region: transpose, stage, AllGather (a full level of slack)
            if l < NL - 1 and Bl > 0:
                transp_blocks(Kl // P, KB // P)
                nc.sync.dma_start(
                    out=agt[Kl:KB, :].rearrange("(blk p) f -> p blk f", p=P),
                    in_=hT[:, Kl:KB].rearrange("p (blk f) -> p blk f", f=P))
                cc = nc.gpsimd.collective_compute(
                    "AllGather", bass.mybir.AluOpType.bypass,
                    replica_groups=RG,
                    ins=[agt[Kl:KB, :].opt()],
                    outs=[h_slab[int(hoff[l]) + NC * Kl:int(hoff[l]) + NC * KB,
                                 :].opt()],
                )
                ccB[l] = cc

            # next level's seeds + MLP head for this level (overlap work)
            if l + 1 < NL:
                emit_seeds(l + 1)
            for g, w in enumerate(widths):
                emit_mlp(l, g, w, hnew[g])

            # next level's shal/deep matmuls + prebuild of fresh one-hots
            if l + 1 < NL:
                ninfo = sched["levels"][l + 1]
                oh_s = emit_onehot(ninfo, "shal")
                emit_chunks(l + 1, "shal", Hg["shal"], oh_s)
                oh_d = emit_onehot(ninfo, "deep")
                emit_chunks(l + 1, "deep", Hg["deep"], oh_d)
                Oh["fresh"] = emit_onehot(ninfo, "fresh")
            Hg["deep"] = Hg_deep_next

        for pl in (rpool, tpool, qpool, ppool, hpool, opool, gpool, spool,
                   cpool):
            pl.release()

    nc.compile()
    return nc


# ---------------------------------------------------------------------------
# Entry point
# ---------------------------------------------------------------------------

def _run(inputs, trace=False, reps=1):
    from concourse.bass_utils import run_bass_kernel_spmd

    fl = np.asarray(inputs["forward_level"])
    num_levels = int(fl.max()) + 1
    sched = _preprocess(fl, inputs["edge_index"], num_levels)
    wmat, iota512, vcols_f, vcols_b, vr2, pred0 = _prep_weights(inputs)

    key = (sched["N"], sched["TCg"], sched["TCm"], sched["sumVc"], reps,
           tuple(int(x) for x in sched["Vc"]),
           tuple(int(x) for x in sched["K"]),
           tuple(int(x) for x in sched["B"]),
           tuple((len(i["shal_chunks"]), len(i["deep_chunks"]),
                  len(i["fresh_chunks"])) for i in sched["levels"]))
    if key not in _COMPILED:
        _COMPILED[key] = _build(sched, reps=reps)
    nc = _COMPILED[key]

    import ml_dtypes
    bf16 = ml_dtypes.bfloat16
    in_maps = []
    for c in range(NC):
        nd = np.stack([sched["n0row"][c], sched["degrow"][c]], axis=0)
        in_maps.append({
            "wmat": wmat, "iota512": iota512, "vcolsf": vcols_f,
            "vcolsb": vcols_b, "vr2": vr2,
            "n0deg": nd.astype(bf16),
            "idxs": sched["idxs"][c],
            "ranks": sched["ranks"][c],
        })

    res = run_bass_kernel_spmd(nc, in_maps, core_ids=list(range(NC)),
                               trace=trace)

    nodes_arr = sched["nodes_arr"]
    out = np.zeros(sched["N"], np.float32)
    for c in range(NC):
        oc = res.results[c]["pred"]
        m = nodes_arr[c] >= 0
        out[nodes_arr[c][m]] = oc[m]
    out[fl == 0] = pred0
    return out[:, None], res


def kernel(**inputs):
    out, _ = _run(inputs, trace=False)
    return out


# revision 43
# speedup vs baseline: 2.5329x; 1.0506x over previous
"""DeepSAT GNN message-passing kernel for 8 Trainium2 NeuronCores.

Algorithm notes (validated numerically against the reference):
  - Every node is updated exactly once, at step l = forward_level (levels
    1..19; level-0 nodes keep h0 forever, and their MLP output is a single
    host-computed constant). At update time the node's own hidden state is
    still h0, so the GRU "hidden side" gates are constant vectors
    computable on the host.
  - msg_i = W @ (S_i + n0_i*h0) + deg_i*b, where S_i sums h over "active"
    in-edges (source level in [1, level_i)), n0_i counts inactive in-edges
    and deg_i all in-edges. With u = W^-1 b this folds to
    msg_i = W @ S'_i,  S'_i = S_i + n0_i*h0 + deg_i*u, so the per-gate
    input is  gi_g = (wih_g @ W) @ S'_i + bih_g  -- one fused matmul.
  - Per level, each core's nodes are ordered [sources of next-level
    "fresh" edges | sources of deeper "old" edges | never-consumed rows].
    Only the first two regions are exchanged: a small "fresh" AllGather
    (on the critical path, dispatched right after GRU group 0) and a
    "bulk" AllGather with a full level of slack. Never-consumed rows stay
    local. The whole h pipeline runs in bf16; PSUM stays fp32.
  - Gathers of h[src] split into three phases by source level: "fresh"
    (src = dst_level-1, needs the latest fresh AllGather), "shal"
    (src = dst_level-2, issued one level early), "deep" (src <=
    dst_level-3, issued two levels early) - this spreads the serial SWDGE
    cost of indirect DMAs across the pipeline.
"""

import os
import sys
import numpy as np

sys.path.insert(0, "/opt/trn_rl_repo")

P = 128
D = 128
NC = 8
GW = 512  # psum group width (one bank of fp32)

_COMPILED = {}

PHASES = ("shal", "deep", "fresh")  # "last"-priority order: fresh wins


# ---------------------------------------------------------------------------
# Host-side preprocessing
# ---------------------------------------------------------------------------

def _preprocess(forward_level, edge_index, num_levels):
    fl = np.asarray(forward_level).astype(np.int64)
    ei = np.asarray(edge_index).astype(np.int64)
    src, dst = ei[0], ei[1]
    N = fl.shape[0]
    NL = num_levels

    lv_s, lv_d = fl[src], fl[dst]
    act = (lv_s >= 1) & (lv_s < lv_d)
    ae = np.where(act)[0]
    fresh_needed = np.zeros(N, bool)
    old_needed = np.zeros(N, bool)
    fe = lv_d[ae] == lv_s[ae] + 1
    fresh_needed[src[ae[fe]]] = True
    old_needed[src[ae[~fe]]] = True

    deg = np.bincount(dst, minlength=N).astype(np.float64)
    n0 = np.bincount(dst[~act], minlength=N).astype(np.float64)

    # --- per level: deal nodes to cores round-robin per category, order
    # [fresh | old | rest] within each core, pad regions to K/B/Vc ---
    K = np.zeros(NL, np.int64)
    B = np.zeros(NL, np.int64)
    Vc = np.zeros(NL, np.int64)
    hoff = np.zeros(NL + 1, np.int64)
    hoff[1] = 1  # slab row 0 is the zeroed dummy row for padded edge lanes
    core_nodes = [[None] * NL for _ in range(NC)]  # per (core, level): node ids

    def rup(x, m):
        return ((x + m - 1) // m) * m

    for l in range(NL):
        nodes_l = np.where(fl == l)[0]
        if l == 0:
            # level 0 never runs on the device and is never a source
            Vc[l] = 0
            hoff[1] = 1
            for c in range(NC):
                core_nodes[c][l] = np.empty(0, np.int64)
            continue
        fr = nodes_l[fresh_needed[nodes_l]]
        on = nodes_l[old_needed[nodes_l] & ~fresh_needed[nodes_l]]
        rs = nodes_l[~fresh_needed[nodes_l] & ~old_needed[nodes_l]]
        Kl = rup(max(len(fr[c::NC]) for c in range(NC)), P) if len(fr) else 0
        Bl = rup(max(len(on[c::NC]) for c in range(NC)), P) if len(on) else 0
        assert Kl <= GW, f"fresh region {Kl} exceeds one psum group"
        vc = rup(Kl + Bl + max(len(rs[c::NC]) for c in range(NC)), P)
        vc = max(vc, P)
        K[l], B[l], Vc[l] = Kl, Bl, vc
        for c in range(NC):
            core_nodes[c][l] = (fr[c::NC], on[c::NC], rs[c::NC])

    for l in range(1, NL):
        hoff[l + 1] = hoff[l] + NC * (K[l] + B[l])
    SLAB = int(hoff[NL])

    Voff = np.zeros(NL + 1, np.int64)
    Voff[1:] = np.cumsum(Vc)
    sumVc = int(Voff[NL])

    # node -> (core, pos-within-core-level) and slab index
    node_pos = np.full(N, -1, np.int64)
    node_core = np.full(N, -1, np.int64)
    srcslab = np.full(N, -1, np.int64)
    nodes_arr = np.full((NC, sumVc), -1, np.int64)
    n0row = np.zeros((NC, sumVc), np.float32)
    degrow = np.zeros((NC, sumVc), np.float32)
    for l in range(1, NL):
        for c in range(NC):
            fr, on, rs = core_nodes[c][l]
            pos = np.concatenate([
                np.arange(len(fr)),
                int(K[l]) + np.arange(len(on)),
                int(K[l] + B[l]) + np.arange(len(rs)),
            ])
            nd = np.concatenate([fr, on, rs])
            node_pos[nd] = pos
            node_core[nd] = c
            slab = np.where(
                pos < K[l], hoff[l] + c * K[l] + pos,
                hoff[l] + NC * K[l] + c * B[l] + (pos - K[l]))
            srcslab[nd[pos < K[l] + B[l]]] = slab[pos < K[l] + B[l]]
            nodes_arr[c, Voff[l] + pos] = nd
            n0row[c, Voff[l] + pos] = n0[nd]
            degrow[c, Voff[l] + pos] = deg[nd]

    # --- active edge table, split by phase ---
    er = ae
    e_lvl = lv_d[er]
    e_src = src[er]
    e_dst = dst[er]
    assert (srcslab[e_src] >= 0).all()
    e_srcslab = srcslab[e_src]
    e_core = node_core[e_dst]
    e_wl = node_pos[e_dst]
    gap = e_lvl - lv_s[er]  # 1 = fresh, 2 = shal, >=3 = deep

    # psum groups per level: the fresh region [0, K) is its own (small)
    # group so the critical fresh AllGather only waits on a K-wide GRU;
    # the rest is split into 512-wide groups
    gstarts = []
    for l in range(NL):
        gs = [0]
        x = int(K[l]) if K[l] > 0 else GW
        while x < int(Vc[l]):
            gs.append(x)
            x += GW
        gstarts.append(np.array(gs, np.int64))
    e_grp = np.zeros(len(er), np.int64)
    for l in range(1, NL):
        m = e_lvl == l
        e_grp[m] = np.searchsorted(gstarts[l], e_wl[m], side="right") - 1

    levels = []
    gcol = 0
    mcol = 0
    idx_cols = [[] for _ in range(NC)]
    rank_cols = [[] for _ in range(NC)]
    for l in range(NL):
        gst = gstarts[l]
        gw = np.diff(np.append(gst, int(Vc[l]))).astype(np.int64)
        info = {"ngrp": len(gst), "gstart": gst, "gw": gw}
        for ph in PHASES:
            info[ph + "_chunks"] = []
            info[ph + "_g"] = (gcol, 0)
        if l >= 1:
            in_lvl = e_lvl == l
            for ph, sel0 in (("shal", in_lvl & (gap == 2)),
                             ("deep", in_lvl & (gap >= 3)),
                             ("fresh", in_lvl & (gap == 1))):
                g_start = gcol
                chunks = []
                percore = [np.where(sel0 & (e_core == c))[0] for c in range(NC)]
                percore = [es[np.argsort(e_wl[es], kind="stable")]
                           for es in percore]
                nch = (max(len(es) for es in percore) + P - 1) // P
                for ch in range(nch):
                    lanes = [es[ch * P:(ch + 1) * P] for es in percore]
                    for c in range(NC):
                        iv = np.zeros(P, np.int32)
                        iv[: len(lanes[c])] = e_srcslab[lanes[c]]
                        idx_cols[c].append(iv)
                    groups = sorted(set(
                        int(g) for c in range(NC) for g in e_grp[lanes[c]]))
                    for g in groups:
                        # narrow the one-hot / matmul to the slot range this
                        # chunk actually touches within the group (edges are
                        # slot-sorted, so the range is tight)
                        gs = int(gst[g])
                        slots = np.concatenate([
                            e_wl[lanes[c]][e_grp[lanes[c]] == g]
                            for c in range(NC)])
                        lo = int(slots.min()) - gs
                        wd = min(int(slots.max()) - gs - lo + 1, GW)
                        for c in range(NC):
                            rv = np.full(P, -1.0, np.float16)
                            es = lanes[c]
                            m = e_grp[es] == g
                            rv[: len(es)][m] = (e_wl[es[m]] - gs - lo
                                                ).astype(np.float16)
                            rank_cols[c].append(rv)
                        chunks.append((g, lo, wd, gcol, mcol))
                        mcol += 1
                    gcol += 1
                info[ph + "_chunks"] = chunks
                info[ph + "_g"] = (g_start, gcol - g_start)
        levels.append(info)

    TCg = max(gcol, 1)
    TCm = max(mcol, 1)
    idxs = np.zeros((NC, P, TCg), np.int32)
    ranks = np.full((NC, P, TCm), -1.0, np.float16)
    for c in range(NC):
        if idx_cols[c]:
            idxs[c, :, :gcol] = np.stack(idx_cols[c], axis=1)
            ranks[c, :, :mcol] = np.stack(rank_cols[c], axis=1)

    # stop-flag bookkeeping: last matmul per (level, grp); PHASES order
    # matches device emission order (shal, deep, then fresh at level start)
    for l in range(NL):
        info = levels[l]
        last = {}
        for ph in PHASES:
            for (grp, lo, wd, g0, m0) in info[ph + "_chunks"]:
                last[grp] = (ph, m0)
        info["last"] = last

    return {
        "N": N, "NL": NL, "K": K, "B": B, "Vc": Vc, "Voff": Voff,
        "hoff": hoff, "SLAB": SLAB, "sumVc": sumVc,
        "TCg": TCg, "TCm": TCm,
        "levels": levels, "idxs": idxs, "ranks": ranks,
        "n0row": n0row, "degrow": degrow, "nodes_arr": nodes_arr,
        "fl": fl,
    }


def _prep_weights(inp):
    import ml_dtypes
    bf16 = ml_dtypes.bfloat16
    f64 = np.float64
    W = inp["aggr_w"].astype(f64)
    b = inp["aggr_b"].astype(f64)
    h0 = (inp["emd_w"][:, 0] + inp["emd_b"]).astype(f64)
    wih = inp["gru_wih"].astype(f64)
    whh = inp["gru_whh"].astype(f64)
    bih = inp["gru_bih"].astype(f64)
    bhh = inp["gru_bhh"].astype(f64)
    u = np.linalg.solve(W, b)
    assert np.abs(W @ u - b).max() < 1e-6
    ghc = whh @ h0 + bhh
    hr_c, hz_c, hn_c = ghc[:D], ghc[D:2 * D], ghc[2 * D:]
    bih_r, bih_z, bih_n = bih[:D], bih[D:2 * D], bih[2 * D:]
    WgT = [(wih[g * D:(g + 1) * D] @ W).T for g in range(3)]

    W1 = inp["w1"].astype(f64)  # [256, 128]
    W2 = inp["w2"].astype(f64)  # [256, 256]
    w3 = inp["w3"].astype(f64)  # [1, 256]
    assert W1.shape[0] == 256

    blocks = [
        WgT[0], WgT[1], WgT[2], np.diag(hn_c),
        W1[0:128, :].T, W1[128:256, :].T,
        W2[0:128, 0:128].T, W2[0:128, 128:256].T,
        W2[128:256, 0:128].T, W2[128:256, 128:256].T,
        np.eye(128),
    ]
    wmat = np.concatenate(blocks, axis=1).astype(bf16)  # [128, 11*128] bf16

    iota512 = np.tile(np.arange(GW, dtype=np.float16)[None, :], (P, 1))

    vcols = np.stack([
        h0,                      # 0: h0 column
        -h0,                     # 1: -h0 (scalar-engine bias for t3)
        bih_r + hr_c,            # 2: sigmoid bias for r
        -(bih_z + hz_c),         # 3: sigmoid bias for z' (scale = -1)
        bih_n,                   # 4: tanh bias for n
        inp["b1"].astype(f64)[0:128],    # 5
        inp["b1"].astype(f64)[128:256],  # 6
        inp["b2"].astype(f64)[0:128],    # 7
        inp["b2"].astype(f64)[128:256],  # 8
        w3[0, 0:128],            # 9
        w3[0, 128:256],          # 10
        np.full(128, inp["b3"].astype(f64)[0]),  # 11: b3 (row 0 used)
    ], axis=1)
    vcols_f = vcols.astype(np.float32)   # [128, 12]
    vcols_b = vcols.astype(bf16)         # [128, 12]

    vr2 = np.stack([h0, u], axis=0).astype(bf16)  # [2, 128]

    # fp8 DoubleRow packs for the K=256 MLP layers: one instruction sums
    # both K-halves at 0.5 cycles/row. Layout [128, 2, M] per m-half.
    f8 = ml_dtypes.float8_e4m3
    w2dr = np.concatenate([
        W2[0:128, 0:128].T, W2[0:128, 128:256].T,      # m0: k0 | k1
        W2[128:256, 0:128].T, W2[128:256, 128:256].T,  # m1: k0 | k1
    ], axis=1).astype(f8)                              # [128, 512]
    w3dr = np.stack([w3[0, 0:128], w3[0, 128:256]], axis=1).astype(f8)  # [128,2]

    # level-0 output: MLP(h0), identical for every level-0 node
    z1 = np.maximum(W1 @ h0 + inp["b1"].astype(f64), 0.0)
    z2 = np.maximum(W2 @ z1 + inp["b2"].astype(f64), 0.0)
    pred0 = float(w3[0] @ z2 + inp["b3"].astype(f64)[0])

    return wmat, iota512, vcols_f, vcols_b, vr2, w2dr, w3dr, pred0


# ---------------------------------------------------------------------------
# Bass program
# ---------------------------------------------------------------------------

WM = {name: i for i, name in enumerate(
    ["WgT_r", "WgT_z", "WgT_n", "diag_hn", "W1Ta", "W1Tb",
     "W2_k0m0", "W2_k1m0", "W2_k0m1", "W2_k1m1", "ident"])}
VC = {name: i for i, name in enumerate(
    ["h0", "nh0", "bias_r", "nbias_z", "bias_n", "b1a", "b1b", "b2a", "b2b",
     "w3a", "w3b", "b3"])}


def _build(sched, reps=1):
    import concourse.bacc as bacc
    import concourse.tile as tile
    from concourse import bass, mybir

    f32 = mybir.dt.float32
    f16 = mybir.dt.float16
    bf16 = mybir.dt.bfloat16
    f8 = mybir.dt.float8e4
    i32 = mybir.dt.int32
    DR = mybir.MatmulPerfMode.DoubleRow
    AF = mybir.ActivationFunctionType
    OP = mybir.AluOpType
    NL = sched["NL"]
    K = sched["K"]
    B = sched["B"]
    Vc = sched["Vc"]
    Voff = sched["Voff"]
    hoff = sched["hoff"]
    SLAB = sched["SLAB"]
    TCg = sched["TCg"]
    TCm = sched["TCm"]
    sumVc = sched["sumVc"]
    Vmax = int(Vc.max())
    KBmax = int((K + B).max())
    RG = [list(range(NC))]

    nc = bacc.Bacc("TRN2", target_bir_lowering=False, debug=False,
                   enable_asserts=False, num_devices=NC)

    wmat_d = nc.dram_tensor("wmat", [P, P * len(WM)], bf16, kind="ExternalInput")
    iota_d = nc.dram_tensor("iota512", [P, GW], f16, kind="ExternalInput")
    vcf_d = nc.dram_tensor("vcolsf", [P, len(VC)], f32, kind="ExternalInput")
    vcb_d = nc.dram_tensor("vcolsb", [P, len(VC)], bf16, kind="ExternalInput")
    vr2_d = nc.dram_tensor("vr2", [2, P], bf16, kind="ExternalInput")
    w2dr_d = nc.dram_tensor("w2dr", [P, 4 * P], f8, kind="ExternalInput")
    w3dr_d = nc.dram_tensor("w3dr", [P, 2], f8, kind="ExternalInput")
    nd_d = nc.dram_tensor("n0deg", [2, sumVc], bf16, kind="ExternalInput")
    idx_d = nc.dram_tensor("idxs", [P, TCg], i32, kind="ExternalInput")
    rnk_d = nc.dram_tensor("ranks", [P, TCm], f16, kind="ExternalInput")
    pred_d = nc.dram_tensor("pred", [sumVc], f32, kind="ExternalOutput")
    h_slab = nc.dram_tensor("h_slab", [SLAB, D], bf16, kind="Internal",
                            addr_space="Shared")
    ag_in = [nc.dram_tensor(f"ag_in{i}", [max(KBmax, P), D], bf16,
                            kind="Internal") for i in range(2)]

    with tile.TileContext(nc) as tc:
        cpool = tc.alloc_tile_pool(name="const", bufs=1)
        spool = tc.alloc_tile_pool(name="sbuf", bufs=2)
        gpool = tc.alloc_tile_pool(name="gath", bufs=2)
        opool = tc.alloc_tile_pool(name="oneh", bufs=24)
        hpool = tc.alloc_tile_pool(name="hnew", bufs=6)
        # 4 S banks (fresh group + 3 bulk groups); transposes reuse the S
        # ring (each group's transpose lands in the bank its S just left)
        ppool = tc.alloc_tile_pool(name="psS", bufs=4, space="PSUM")
        qpool = tc.alloc_tile_pool(name="psG", bufs=3, space="PSUM")
        rpool = tc.alloc_tile_pool(name="psP", bufs=1, space="PSUM")

        # ---- load constants ----
        wm = cpool.tile([P, P * len(WM)], bf16, tag="wm")
        nc.sync.dma_start(out=wm[:], in_=wmat_d[:])
        iota = cpool.tile([P, GW], f16, tag="iota")
        nc.sync.dma_start(out=iota[:], in_=iota_d[:])
        vcf = cpool.tile([P, len(VC)], f32, tag="vcf")
        nc.sync.dma_start(out=vcf[:], in_=vcf_d[:])
        vcb = cpool.tile([P, len(VC)], bf16, tag="vcb")
        nc.sync.dma_start(out=vcb[:], in_=vcb_d[:])
        vr2 = cpool.tile([2, P], bf16, tag="vr2")
        nc.sync.dma_start(out=vr2[:], in_=vr2_d[:])
        w2dr = cpool.tile([P, 4 * P], f8, tag="w2dr")
        nc.sync.dma_start(out=w2dr[:], in_=w2dr_d[:])
        w3dr = cpool.tile([P, 2], f8, tag="w3dr")
        nc.sync.dma_start(out=w3dr[:], in_=w3dr_d[:])
        idxs = cpool.tile([P, TCg], i32, tag="idxs")
        nc.sync.dma_start(out=idxs[:], in_=idx_d[:])
        rnks = cpool.tile([P, TCm], f16, tag="rnks")
        nc.sync.dma_start(out=rnks[:], in_=rnk_d[:])

        def wmb(name):
            return wm[:, WM[name] * P:(WM[name] + 1) * P]

        def vf(name):
            return vcf[:, VC[name]:VC[name] + 1]

        def vb(name):
            return vcb[:, VC[name]:VC[name] + 1]

        # zero slab row 0: padded edge lanes gather it, and while their
        # all-zero one-hot column discards the value, 0 * NaN = NaN if the
        # row were left as uninitialized DRAM
        zrow = cpool.tile([1, D], bf16, tag="zrow")
        nc.vector.memset(zrow[:], 0.0)
        zrow_dma = nc.sync.dma_start(out=h_slab[0:1, :], in_=zrow[:])

        # ---- per-level state ----
        S_ps = [None] * NL
        ccF = {}   # level -> fresh AllGather
        ccB = {}   # level -> bulk AllGather

        def grp_geom(l):
            info = sched["levels"][l]
            return [(int(s), int(w))
                    for s, w in zip(info["gstart"], info["gw"])]

        def emit_onehot(info, ph):
            chunks = info[ph + "_chunks"]
            if not chunks:
                return None
            ohs = []
            for (grp, lo, wd, gcol, mcol) in chunks:
                oh = opool.tile([P, GW], bf16, tag="oh" + ph)
                nc.vector.tensor_tensor(
                    out=oh[:, :wd],
                    in0=rnks[:, mcol:mcol + 1].to_broadcast([P, wd]),
                    in1=iota[:, :wd],
                    op=OP.is_equal,
                )
                ohs.append(oh)
            return ohs

        def emit_gather(info, ph, bound, deps):
            """Gather h rows for one phase; reads only h_slab[0:bound]."""
            g0, k = info[ph + "_g"]
            if k == 0:
                return None
            hg = gpool.tile([P, k * D], bf16, tag="hg_" + ph)
            for j in range(k):
                gi = nc.gpsimd.indirect_dma_start(
                    out=hg[:, j * D:(j + 1) * D],
                    out_offset=None,
                    in_=h_slab[0:bound, :],
                    in_offset=bass.IndirectOffsetOnAxis(
                        ap=idxs[:, g0 + j:g0 + j + 1], axis=0),
                )
                # dynamic reads of h_slab are not region-tracked by Tile's
                # shadow memory: pin the RAW edges by hand
                for dep in deps:
                    if dep is not None:
                        tile.add_dep_helper(gi.ins, dep.ins, sync=True,
                                            reason="gather reads AllGather out")
                tile.add_dep_helper(gi.ins, zrow_dma.ins, sync=True,
                                    reason="gather may read zeroed row 0")
            return hg

        def emit_seeds(l):
            """allocate S psums for level l, seed with h0 (x) n0 + u (x) deg."""
            tiles = []
            info = sched["levels"][l]
            v = int(Vc[l])
            off = int(Voff[l])
            ndr = spool.tile([2, Vmax], bf16, tag="ndr")
            nc.sync.dma_start(out=ndr[:, :v], in_=nd_d[:, off:off + v])
            for g, (gs, w) in enumerate(grp_geom(l)):
                sp = ppool.tile([P, GW], f32, tag="S", space="PSUM")
                is_last = info["last"].get(g) is None
                nc.tensor.matmul(
                    out=sp[:, :w], lhsT=vr2[:],
                    rhs=ndr[:, gs:gs + w],
                    start=True, stop=is_last, skip_group_check=True)
                tiles.append(sp)
            S_ps[l] = tiles

        def emit_chunks(l, ph, hg, ohs):
            info = sched["levels"][l]
            chunks = info[ph + "_chunks"]
            if not chunks:
                return
            g0 = info[ph + "_g"][0]
            for i, (grp, lo, wd, gcol, mcol) in enumerate(chunks):
                j = gcol - g0
                is_last = info["last"].get(grp) == (ph, mcol)
                nc.tensor.matmul(
                    out=S_ps[l][grp][:, lo:lo + wd],
                    lhsT=hg[:, j * D:(j + 1) * D],
                    rhs=ohs[i][:, :wd],
                    start=False, stop=is_last, skip_group_check=True)

        def emit_mlp(l, lo, w, rhs_sb):
            """MLP head for one 512-wide node window; writes pred rows."""
            z1s = []
            for half in ("a", "b"):
                zp = qpool.tile([P, GW], f32, tag="G", space="PSUM")
                nc.tensor.matmul(out=zp[:, :w], lhsT=wmb("W1T" + half),
                                 rhs=rhs_sb[:, :w], start=True, stop=True)
                zs = spool.tile([P, GW], bf16, tag="z1" + half)
                nc.scalar.activation(out=zs[:, :w], in_=zp[:, :w],
                                     func=AF.Relu, bias=vf("b1" + half))
                z1s.append(zs)
            z2s = []
            for mi, mh in enumerate(("m0", "m1")):
                zp = qpool.tile([P, GW], f32, tag="G", space="PSUM")
                nc.tensor.matmul(out=zp[:, :w], lhsT=wmb("W2_k0" + mh),
                                 rhs=z1s[0][:, :w], start=True, stop=False)
                nc.tensor.matmul(out=zp[:, :w], lhsT=wmb("W2_k1" + mh),
                                 rhs=z1s[1][:, :w], start=False, stop=True)
                zs = spool.tile([P, GW], bf16, tag="z2" + mh)
                nc.vector.tensor_scalar(out=zs[:, :w], in0=zp[:, :w],
                                        scalar1=vf("b2" + ("a" if mi == 0 else "b")),
                                        scalar2=0.0, op0=OP.add, op1=OP.max)
                z2s.append(zs)
            pp = rpool.tile([1, GW], f32, tag="pred", space="PSUM")
            nc.tensor.matmul(out=pp[:, :w], lhsT=vb("w3a"), rhs=z2s[0][:, :w],
                             start=True, stop=False)
            nc.tensor.matmul(out=pp[:, :w], lhsT=vb("w3b"), rhs=z2s[1][:, :w],
                             start=False, stop=True)
            ps = spool.tile([1, GW], f32, tag="psb")
            nc.scalar.activation(out=ps[:, :w], in_=pp[:, :w], func=AF.Identity,
                                 bias=vcf[0:1, VC["b3"]:VC["b3"] + 1])
            off = int(Voff[l]) + lo
            nc.sync.dma_start(out=pred_d[off:off + w], in_=ps[0:1, :w])

        def emit_gru(l, g, gs, w, hnewt):
            """GRU for one psum group; writes hnewt[:, gs:gs+w]."""
            ssb = spool.tile([P, GW], bf16, tag="Ssb")
            nc.vector.tensor_copy(out=ssb[:, :w], in_=S_ps[l][g][:, :w])

            gr = qpool.tile([P, GW], f32, tag="G", space="PSUM")
            nc.tensor.matmul(out=gr[:, :w], lhsT=wmb("WgT_r"),
                             rhs=ssb[:, :w], start=True, stop=True)
            gz = qpool.tile([P, GW], f32, tag="G", space="PSUM")
            nc.tensor.matmul(out=gz[:, :w], lhsT=wmb("WgT_z"),
                             rhs=ssb[:, :w], start=True, stop=True)
            gn = qpool.tile([P, GW], f32, tag="G", space="PSUM")
            nc.tensor.matmul(out=gn[:, :w], lhsT=wmb("WgT_n"),
                             rhs=ssb[:, :w], start=True, stop=False)

            rsb = spool.tile([P, GW], bf16, tag="rsb")
            nc.scalar.activation(out=rsb[:, :w], in_=gr[:, :w],
                                 func=AF.Sigmoid, bias=vf("bias_r"))
            zsb = spool.tile([P, GW], bf16, tag="zsb")
            nc.scalar.activation(out=zsb[:, :w], in_=gz[:, :w],
                                 func=AF.Sigmoid, bias=vf("nbias_z"),
                                 scale=-1.0)
            nc.tensor.matmul(out=gn[:, :w], lhsT=wmb("diag_hn"),
                             rhs=rsb[:, :w], start=False, stop=True)
            nsb = spool.tile([P, GW], bf16, tag="nsb")
            nc.scalar.activation(out=nsb[:, :w], in_=gn[:, :w],
                                 func=AF.Tanh, bias=vf("bias_n"))

            # t3 = n - h0 on the scalar engine (keeps DVE free)
            t3 = spool.tile([P, GW], bf16, tag="t3")
            nc.scalar.activation(out=t3[:, :w], in_=nsb[:, :w],
                                 func=AF.Identity, bias=vf("nh0"))
            t4 = spool.tile([P, GW], bf16, tag="t4")
            nc.vector.tensor_tensor(out=t4[:, :w], in0=t3[:, :w],
                                    in1=zsb[:, :w], op=OP.mult)
            nc.scalar.activation(out=hnewt[:, gs:gs + w], in_=t4[:, :w],
                                 func=AF.Identity, bias=vf("h0"))

        # reps>1 repeats the whole computation for wall-clock timing: the
        # computation is idempotent (h_slab/pred rewritten with same values)
        for _rep in range(reps):
          emit_seeds(1)
          Oh = {ph: None for ph in PHASES}
          Hg = {ph: None for ph in PHASES}  # most recent gather per phase
          Hg_deep_next = None               # deep gather issued 2 levels early

          # ================= levels 1..NL-1 =================
          for l in range(1, NL):
            info = sched["levels"][l]
            geom = grp_geom(l)
            ngrp = len(geom)
            Kl, Bl = int(K[l]), int(B[l])
            KB = Kl + Bl

            # fresh gather + chunks for this level (critical path; one-hots
            # prebuilt during the previous level)
            hg_f = emit_gather(info, "fresh",
                               int(hoff[l - 1] + NC * K[l - 1]) if l >= 2 else 1,
                               [ccF.get(l - 1)])
            emit_chunks(l, "fresh", hg_f, Oh["fresh"])

            # GRU of group 0 (= the K-wide fresh region): its rows are
            # transposed, staged and AllGathered before the other groups run
            hnewt = hpool.tile([P, Vmax], bf16, tag="hnew")
            emit_gru(l, 0, geom[0][0], geom[0][1], hnewt)

            hT = None
            agt = ag_in[l % 2]

            def transp_blocks(lo_blk, hi_blk):
                """Transpose hnewt 128-blocks [lo, hi) via the S psum ring
                (one tile per psum group, reusing the bank its S vacated)."""
                cur_g = None
                tp = None
                lo_c = None
                hi_c = None

                def flush():
                    nc.vector.tensor_copy(out=hT[:, lo_c:hi_c],
                                          in_=tp[:, 0:hi_c - lo_c])

                for blk in range(lo_blk, hi_blk):
                    col = blk * P
                    g = max(i for i, (gs, gw_) in enumerate(geom) if gs <= col)
                    if g != cur_g:
                        if cur_g is not None:
                            flush()
                        tp = ppool.tile([P, GW], bf16, tag="S", space="PSUM")
                        cur_g, lo_c = g, col
                    hi_c = col + P
                    nc.tensor.transpose(
                        out=tp[:, col - lo_c:col - lo_c + P],
                        in_=hnewt[:, col:col + P],
                        identity=wmb("ident"))
                if cur_g is not None:
                    flush()

            if l < NL - 1 and KB > 0:
                hT = spool.tile([P, KBmax], bf16, tag="hT")
                if Kl > 0:
                    transp_blocks(0, Kl // P)
                    nc.sync.dma_start(
                        out=agt[0:Kl, :].rearrange("(blk p) f -> p blk f", p=P),
                        in_=hT[:, 0:Kl].rearrange("p (blk f) -> p blk f", f=P))
                    cc = nc.gpsimd.collective_compute(
                        "AllGather", bass.mybir.AluOpType.bypass,
                        replica_groups=RG,
                        ins=[agt[0:Kl, :].opt()],
                        outs=[h_slab[int(hoff[l]):int(hoff[l]) + NC * Kl, :].opt()],
                    )
                    ccF[l] = cc

            for g in range(1, ngrp):
                emit_gru(l, g, geom[g][0], geom[g][1], hnewt)

            # shallow gathers for l+1 (src level l-1) and deep gathers for
            # l+2 (src <= l-1): both read rows below hoff[l], written by the
            # l-1 AllGathers, so they run during this level's collectives
            if l + 1 < NL:
                ninfo = sched["levels"][l + 1]
                Hg["shal"] = emit_gather(ninfo, "shal", int(hoff[l]),
                                         [ccF.get(l - 1), ccB.get(l - 1)])
            if l + 2 < NL:
                nninfo = sched["levels"][l + 2]
                Hg_deep_next = emit_gather(nninfo, "deep", int(hoff[l]),
                                           [ccF.get(l - 1), ccB.get(l - 1)])

            # bulk region: transpose, stage, AllGather (a full level of slack)
            if l < NL - 1 and Bl > 0:
                transp_blocks(Kl // P, KB // P)
                nc.sync.dma_start(
                    out=agt[Kl:KB, :].rearrange("(blk p) f -> p blk f", p=P),
                    in_=hT[:, Kl:KB].rearrange("p (blk f) -> p blk f", f=P))
                cc = nc.gpsimd.collective_compute(
                    "AllGather", bass.mybir.AluOpType.bypass,
                    replica_groups=RG,
                    ins=[agt[Kl:KB, :].opt()],
                    outs=[h_slab[int(hoff[l]) + NC * Kl:int(hoff[l]) + NC * KB,
                                 :].opt()],
                )
                ccB[l] = cc

            # next level's seeds + MLP head for this level (overlap work;
            # MLP runs over 512-wide windows independent of psum grouping)
            if l + 1 < NL:
                emit_seeds(l + 1)
            for lo in range(0, int(Vc[l]), GW):
                w = min(GW, int(Vc[l]) - lo)
                emit_mlp(l, lo, w, hnewt[:, lo:lo + w])

            # next level's shal/deep matmuls + prebuild of fresh one-hots
            if l + 1 < NL:
                ninfo = sched["levels"][l + 1]
                oh_s = emit_onehot(ninfo, "shal")
                emit_chunks(l + 1, "shal", Hg["shal"], oh_s)
                oh_d = emit_onehot(ninfo, "deep")
                emit_chunks(l + 1, "deep", Hg["deep"], oh_d)
                Oh["fresh"] = emit_onehot(ninfo, "fresh")
            Hg["deep"] = Hg_deep_next

        for pl in (rpool, qpool, ppool, hpool, opool, gpool, spool,
                   cpool):
            pl.release()

    nc.compile()
    return nc


# ---------------------------------------------------------------------------
# Entry point
# ---------------------------------------------------------------------------

def _run(inputs, trace=False, reps=1):
    from concourse.bass_utils import run_bass_kernel_spmd

    fl = np.asarray(inputs["forward_level"])
    num_levels = int(fl.max()) + 1
    sched = _preprocess(fl, inputs["edge_index"], num_levels)
    (wmat, iota512, vcols_f, vcols_b, vr2, w2dr, w3dr,
     pred0) = _prep_weights(inputs)

    key = (sched["N"], sched["TCg"], sched["TCm"], sched["sumVc"], reps,
           tuple(int(x) for x in sched["Vc"]),
           tuple(int(x) for x in sched["K"]),
           tuple(int(x) for x in sched["B"]),
           tuple((len(i["shal_chunks"]), len(i["deep_chunks"]),
                  len(i["fresh_chunks"])) for i in sched["levels"]))
    if key not in _COMPILED:
        _COMPILED[key] = _build(sched, reps=reps)
    nc = _COMPILED[key]

    import ml_dtypes
    bf16 = ml_dtypes.bfloat16
    in_maps = []
    for c in range(NC):
        nd = np.stack([sched["n0row"][c], sched["degrow"][c]], axis=0)
        in_maps.append({
            "wmat": wmat, "iota512": iota512, "vcolsf": vcols_f,
            "vcolsb": vcols_b, "vr2": vr2, "w2dr": w2dr, "w3dr": w3dr,
            "n0deg": nd.astype(bf16),
            "idxs": sched["idxs"][c],
            "ranks": sched["ranks"][c],
        })

    res = run_bass_kernel_spmd(nc, in_maps, core_ids=list(range(NC)),
                               trace=trace)

    nodes_arr = sched["nodes_arr"]
    out = np.zeros(sched["N"], np.float32)
    for c in range(NC):
        oc = res.results[c]["pred"]
        m = nodes_arr[c] >= 0
        out[nodes_arr[c][m]] = oc[m]
    out[fl == 0] = pred0
    return out[:, None], res


def kernel(**inputs):
    out, _ = _run(inputs, trace=False)
    return out
